# revision 19
# baseline (speedup 1.0000x reference)
"""Cross-attention kernel for 8 Trainium2 NeuronCores (SPMD).

Problem: B=4, T_q=T_kv=2048, Q_DIM=1024, KV_DIM=768, H=16, DK=64, fp32.
  q = q_tokens @ Wq.T ; k = kv_tokens @ Wk.T ; v = kv_tokens @ Wv.T
  out = softmax(q k^T / sqrt(DK)) v @ Wo.T

Sharding (8 cores): core c handles batch b=c//2 and head-group hg=c%2
(8 heads, 512 of the 1024 q-dims).  After attention, the pair (2b, 2b+1)
AllGathers the per-head-group attention outputs, then each core runs the
output projection against ITS half of the Wo columns — core c returns
out[b, :, (c%2)*512:(c%2+1)*512] transposed.  The rank-dependent
output-channel split lives entirely in the host-side Wo slice, so the
device program is identical on all cores.

Engine balance (measured): the window is paced by the scalar engine
(256 exps x 1113ns = 285us busy; FD=1024 is the PSUM-bank max, fp32
matmul output is mandatory on TRN2) with the PE in near-lockstep
(QK pair 385ns concurrent via auto row-groups + 2 PV x 215ns + ~1.3
projection-MM drip capacity per iteration).  This version organizes
everything around keeping that exp stream dense:
  - wq/wk are loaded PER HEAD-PAIR (hp-major host layout), so the
    first-exp gate is 2.9MB not 3.5MB, round-robined over all five
    engine DMA queues; V for the first 4 kv-chunks is produced inside
    the DMA-wait window before the first exp.
  - QK score matmuls run under tc.high_priority so the tile scheduler
    never parks them behind dripped projection matmuls (this was worth
    ~160ns/iteration of exp stall in every pop-heavy block).
  - Remainder DMA order matches need-by: xkv-tb1, xkv-tb23, xq-tb1,
    wk-hp123, wq-hp123, xq-tb23, wo.
  - Output projection po(j) accumulates its hp0-2 chunks first (DMA'd
    from the early AllGathers, prefetched), so only the last 2 of 8
    matmuls per chain gate on head-pair 3's per-j exchange; j=3's
    early chunks are prefetched at block start and the final out
    stores fan out over three DMA queues.

Measured state: 403,058ns = ~31us head (first exp; scalar's DMA ring
must drain its input triggers first — a trigger on a full ring blocks
the queue at transfer-completion pace, so scalar only gets triggers
through xkv-tb1) + ~347us window (ACT 288us busy; ~22us j0 idle is
V-production PE capacity + remainder-DMA pacing, ~12us hp3 is po
overload, ~5us at head-pair boundaries from the normalize/exchange
chain) + ~25us tail-to-drain.

Measured dead ends, do not retry: exp FD=2048 (PSUM bank budget: 4
score banks + 2 PV + 2 projection banks is exactly 8; bf16 matmul
output is TRN3-only), PE warm-up matmuls, ACT-table preload, batched
3D strided input DMAs (device-fatal descriptor fault),
reciprocal_approx_fast off partition 0 (silently wrong),
high_priority offset 96 (knots vector/gpsimd block-end chains, 15us
boundary stalls), ex ring < 8 bufs (-65us!), po stores on gpsimd,
po(3) aog prefetch hoisted to j1 with aog bufs 24 (SBUF + pool-ring
conflicts; regressed to 471us with ex=6).  Run-to-run noise is
+-10-20us from chip power states.
"""

import numpy as np

import concourse.bacc as bacc
import concourse.mybir as mybir
import concourse.tile as tile
from concourse import bass_utils

try:
    import ml_dtypes
    _BF16 = ml_dtypes.bfloat16
except ImportError:  # pragma: no cover
    _BF16 = mybir.dt.np(mybir.dt.bfloat16)

N_CORES = 8
P = 128
TQ = 2048
TKV = 2048
CQ = 1024     # q_tokens channels
CKV = 768     # kv_tokens channels
DQ = 512      # per-core head-group q dims (8 heads x 64)
DO = 512      # per-core output channels (half of 1024)
NJ = 4        # 512-wide tq j-blocks (== projection t-blocks)
NI = TKV // P  # 16 kv chunks
NHP = DQ // P  # 4 head-pairs
CQ_CH = CQ // P   # 8
CKV_CH = CKV // P  # 6
NCC = 2 * NHP     # 8 dc chunks in the gathered attention output

F32 = mybir.dt.float32
BF = mybir.dt.bfloat16
EXP = mybir.ActivationFunctionType.Exp
MUL = mybir.AluOpType.mult

_compiled = None


def _build():
    nc = bacc.Bacc("TRN2", target_bir_lowering=False, debug=False,
                   num_devices=N_CORES)

    xqT = nc.dram_tensor("xqT", [CQ, TQ], BF, kind="ExternalInput")
    xkvT = nc.dram_tensor("xkvT", [CKV, TKV], BF, kind="ExternalInput")
    # wq/wk come hp-major from the host: [128, hp, chunk, 128] flattened,
    # so one contiguous transfer delivers exactly one head-pair's slice.
    wqT = nc.dram_tensor("wqT", [P, NHP * CQ_CH * P], BF,
                         kind="ExternalInput")
    wkT = nc.dram_tensor("wkT", [P, NHP * CKV_CH * P], BF,
                         kind="ExternalInput")
    wvT = nc.dram_tensor("wvT", [P, CKV_CH * DQ], BF, kind="ExternalInput")
    # full-dc Wo slice for this core's output-channel half, dc rows in
    # gathered order (head-group 0 rows then head-group 1 rows)
    woT = nc.dram_tensor("woT", [P, NCC * DO], BF, kind="ExternalInput")
    onesc = nc.dram_tensor("onesc", [P, 8], BF, kind="ExternalInput")
    out_ext = nc.dram_tensor("out", [DO, TQ], BF, kind="ExternalOutput")

    groups = [[2 * b, 2 * b + 1] for b in range(N_CORES // 2)]

    with tile.TileContext(nc) as tc:
        with (
            tc.tile_pool(name="weights", bufs=1) as wpool,
            tc.tile_pool(name="xres", bufs=1) as xpool,
            tc.tile_pool(name="kqv", bufs=1) as kpool,
            tc.tile_pool(name="stage", bufs=1) as stpool,
            tc.tile_pool(name="dram", bufs=1, space="DRAM") as dpool,
            tc.tile_pool(name="psum_s", bufs=2, space="PSUM") as ps_s,
            tc.tile_pool(name="psum_pv", bufs=2, space="PSUM") as ps_pv,
            tc.tile_pool(name="psum_pj", bufs=2, space="PSUM") as ps_pj,
        ):
            # ---- resident weights + token inputs (bf16) ----
            wk_sb = wpool.tile([P, NHP, CKV_CH, P], BF, tag="wk")
            wq_sb = wpool.tile([P, NHP, CQ_CH, P], BF, tag="wq")
            wv_sb = wpool.tile([P, CKV_CH, DQ], BF, tag="wv")
            wo_sb = wpool.tile([P, NCC, DO], BF, tag="wo")
            ones_sb = wpool.tile([P, 8, 1], BF, tag="ones")
            xkv_sb = [xpool.tile([P, TKV], BF, tag="xkv", bufs=CKV_CH,
                                 name=f"xkv{c}") for c in range(CKV_CH)]
            xq_sb = [xpool.tile([P, TQ], BF, tag="xq", bufs=CQ_CH,
                                name=f"xq{c}") for c in range(CQ_CH)]

            # All input transfers issued up front on the three DMA-capable
            # queues (SP/gpsimd/ACT — triggers are ~650ns each and all fit
            # in the scalar queue's pre-first-exp idle), in need-by order;
            # per-queue transfers execute in trigger order, so issue order
            # IS arrival order.  Prefix gating the first exp: wk-hp0,
            # xkv-tb0, wv (feeds prelude V), wq-hp0, xq-tb0 = 2.9MB.  Then
            # xkv-tb1 (K(0,1)+V mid-j0), xkv-tb23, xq-tb1 (Q(0,1) by j1),
            # wk/wq hp1-3, xq-tb23 (Q(0,2)+ by j2), wo (head-pair 3).
            q3 = [nc.sync, nc.gpsimd, nc.scalar]
            xfers = [(wk_sb[:, 0], wkT.ap()[:, 0:CKV_CH * P])]
            for c in range(CKV_CH):
                xfers.append((xkv_sb[c][:, 0:512],
                              xkvT.ap()[c * P:(c + 1) * P, 0:512]))
            for c in range(CKV_CH):
                xfers.append((wv_sb[:, c, :],
                              wvT.ap()[:, c * DQ:(c + 1) * DQ]))
            xfers.append((wq_sb[:, 0], wqT.ap()[:, 0:CQ_CH * P]))
            for c in range(CQ_CH):
                xfers.append((xq_sb[c][:, 0:512],
                              xqT.ap()[c * P:(c + 1) * P, 0:512]))
            for c in range(CKV_CH):
                xfers.append((xkv_sb[c][:, 512:1024],
                              xkvT.ap()[c * P:(c + 1) * P, 512:1024]))
            for c in range(CKV_CH):
                xfers.append((xkv_sb[c][:, 1024:TKV],
                              xkvT.ap()[c * P:(c + 1) * P, 1024:TKV]))
            for c in range(CQ_CH):
                xfers.append((xq_sb[c][:, 512:1024],
                              xqT.ap()[c * P:(c + 1) * P, 512:1024]))
            for hp in range(1, NHP):
                xfers.append((wk_sb[:, hp],
                              wkT.ap()[:, hp * CKV_CH * P:(hp + 1) * CKV_CH * P]))
                xfers.append((wq_sb[:, hp],
                              wqT.ap()[:, hp * CQ_CH * P:(hp + 1) * CQ_CH * P]))
            for c in range(CQ_CH):
                xfers.append((xq_sb[c][:, 1024:TQ],
                              xqT.ap()[c * P:(c + 1) * P, 1024:TQ]))
            for cc in range(NCC):
                xfers.append((wo_sb[:, cc, :],
                              woT.ap()[:, cc * DO:(cc + 1) * DO]))
            # ones first (2KB — gates the vt ones-columns for the first PV)
            nc.gpsimd.dma_start(ones_sb[:],
                                onesc.ap().rearrange("p (n o) -> p n o", o=1))
            # A DMA trigger BLOCKS its engine queue while the hardware ring
            # is full, pacing at transfer-completion rate — so the scalar
            # queue only gets triggers that drain before the first exp
            # (~5 x 128-192KB); everything later goes to sync/gpsimd.
            # (three queues through xkv-tb1: scalar's ring drains those
            # triggers before the first exp, and j0's inline K/V work
            # otherwise starves on the 2-queue remainder stream)
            for n, (dst, src) in enumerate(xfers):
                if n < 28:
                    q3[n % 3].dma_start(dst, src)
                else:
                    q3[n % 2].dma_start(dst, src)

            # ---- SBUF-resident K/Q/V (written by projection evictions) ----
            kb = [[kpool.tile([P, 512], BF, tag="kb", bufs=NHP * NJ,
                              name=f"kb{hp}_{tb}") for tb in range(NJ)]
                  for hp in range(NHP)]
            qs = [[kpool.tile([P, 512], BF, tag="qs", bufs=NHP * NJ,
                              name=f"qs{hp}_{tb}") for tb in range(NJ)]
                  for hp in range(NHP)]
            # vt[tc]: [128 tkv-chunk, 8 heads, 64+1] (ones col -> denominator)
            vt = [kpool.tile([P, 8, 65], BF, tag="vt", bufs=NI,
                             name=f"vt{tc}") for tc in range(NI)]
            # normalized attention output per head-pair (exchanged via CC)
            ao = [kpool.tile([P, TQ], BF, tag="ao", bufs=NHP,
                             name=f"ao{hp}") for hp in range(NHP)]

            # ---- internal DRAM for collectives ----
            agi = [dpool.tile([P, TQ], BF, tag=f"agi{h}", name=f"agi{h}")
                   for h in range(NHP - 1)]
            ago = [dpool.tile([2, P, TQ], BF, tag=f"ago{h}", name=f"ago{h}")
                   for h in range(NHP - 1)]
            agi3 = [dpool.tile([P, 512], BF, tag=f"agi3_{j}", name=f"agi3_{j}")
                    for j in range(NJ)]
            ago3 = [dpool.tile([2, P, 512], BF, tag=f"ago3_{j}",
                               name=f"ago3_{j}") for j in range(NJ)]

            # ============ projection work units (one yield per MM) ========
            def k_gen(hp, tb):
                pk = ps_pj.tile([P, 512], F32, tag="pj", name=f"pk_{hp}_{tb}")
                for c in range(CKV_CH):
                    nc.tensor.matmul(pk[:], wk_sb[:, hp, c, :],
                                     xkv_sb[c][:, tb * 512:(tb + 1) * 512],
                                     start=(c == 0), stop=(c == CKV_CH - 1))
                    if c == CKV_CH - 1:
                        # the eviction gates the next head-pair's QK pairs —
                        # keep it ahead of block-end normalize work on the
                        # vector queue
                        with tc.high_priority(offset=24):
                            nc.vector.tensor_copy(kb[hp][tb][:], pk[:])
                    yield

            def q_gen(hp, tb):
                pq = ps_pj.tile([P, 512], F32, tag="pj", name=f"pq_{hp}_{tb}")
                for c in range(CQ_CH):
                    nc.tensor.matmul(pq[:], wq_sb[:, hp, c, :],
                                     xq_sb[c][:, tb * 512:(tb + 1) * 512],
                                     start=(c == 0), stop=(c == CQ_CH - 1))
                    if c == CQ_CH - 1:
                        with tc.high_priority(offset=24):
                            nc.vector.tensor_copy(qs[hp][tb][:], pq[:])
                    yield

            def v_gen(tc_i):
                pv = ps_pj.tile([P, 512], F32, tag="pj", name=f"pv_{tc_i}")
                for c in range(CKV_CH):
                    nc.tensor.matmul(
                        pv[:],
                        xkv_sb[c][:, tc_i * P:(tc_i + 1) * P],
                        wv_sb[:, c, :],
                        start=(c == 0), stop=(c == CKV_CH - 1))
                    if c == CKV_CH - 1:
                        nc.vector.tensor_copy(
                            vt[tc_i][:, :, 0:64],
                            pv[:].rearrange("p (h d) -> p h d", d=64))
                        nc.vector.tensor_copy(vt[tc_i][:, :, 64:65],
                                              ones_sb[:])
                    yield

            def run_all(gen):
                for _ in gen:
                    pass

            # deferred projection work, drip-fed into the attention loop.
            # Order respects need-by times: Q(hp,tb) before block (hp,tb)
            # starts, K(hp) fully before head-pair hp starts.
            deferred = [
                q_gen(0, 2),
                k_gen(1, 0), k_gen(1, 1),
                q_gen(0, 3),
                k_gen(1, 2), k_gen(1, 3),
                q_gen(1, 0), q_gen(1, 1),
                k_gen(2, 0), k_gen(2, 1), k_gen(2, 2), k_gen(2, 3),
                q_gen(1, 2), q_gen(1, 3),
                q_gen(2, 0),
                k_gen(3, 0), k_gen(3, 1),
                q_gen(2, 1), q_gen(2, 2),
                k_gen(3, 2), k_gen(3, 3),
                q_gen(2, 3),
                q_gen(3, 0), q_gen(3, 1), q_gen(3, 2), q_gen(3, 3),
            ]
            deferred.reverse()

            def pop_work(queue, n):
                while n > 0 and queue:
                    gen = queue[-1]
                    try:
                        next(gen)
                        n -= 1
                    except StopIteration:
                        queue.pop()

            # ================= prelude =================
            # K(0,0) + V(0..3) + Q(0,0).  V production fills the PE-idle
            # DMA-wait window (wv+xkv-tb0 arrive well before wq/xq-tb0);
            # the scheduler lets Q00 overtake any V matmul still waiting
            # on data.  K(0,1..3) and V(4..15) are produced inside j0's
            # iterations as their chunks land; Q(0,1) late in j0.
            run_all(k_gen(0, 0))
            for t in range(4):
                run_all(v_gen(t))
            run_all(q_gen(0, 0))
            k0 = {2: [k_gen(0, 1)], 6: [k_gen(0, 2)], 10: [k_gen(0, 3)]}
            q01 = [q_gen(0, 1)]

            # ============== out-projection work units ==============
            # Chunk order per chain: head-pairs 0-2 first (gathered long
            # ago, DMA'd with no wait), head-pair 3's pair last so only 2
            # of 8 matmuls gate on the final per-j exchange.
            po_order = [(g, hpx) for hpx in range(NHP) for g in range(2)]
            # scalar stays exp-only until the tail, and gpsimd carries the
            # collectives + partition broadcasts — po traffic goes on sync
            # (j=3's stores can use scalar once the exp stream has drained)
            ld_rot = [nc.sync, nc.sync]

            def po_loads(j, pre):
                js = slice(j * 512, (j + 1) * 512)
                for n, (g, hpx) in enumerate(po_order):
                    if hpx == NHP - 1:
                        continue
                    aog = stpool.tile([P, 512], BF, tag="aog", bufs=16,
                                      name=f"aog_{j}_{g}_{hpx}")
                    ld_rot[n % 2].dma_start(aog[:], ago[hpx][g, :, js])
                    pre[n] = aog

            def po_gen(j, pre):
                tail = j == NJ - 1
                for n, (g, hpx) in enumerate(po_order):
                    if hpx == NHP - 1:
                        aog = stpool.tile([P, 512], BF, tag="aog", bufs=16,
                                          name=f"aog_{j}_{g}_{hpx}")
                        eng = nc.scalar if tail else ld_rot[n % 2]
                        eng.dma_start(aog[:], ago3[j][g, :, :])
                        pre[n] = aog
                js = slice(j * 512, (j + 1) * 512)
                for do in range(DO // P):
                    po = ps_pj.tile([P, 512], F32, tag="pj",
                                    name=f"po_{j}_{do}")
                    for n, (g, hpx) in enumerate(po_order):
                        cc = g * NHP + hpx
                        nc.tensor.matmul(
                            po[:], wo_sb[:, cc, do * P:(do + 1) * P],
                            pre[n][:],
                            start=(n == 0), stop=(n == NCC - 1))
                        yield
                    ost = stpool.tile([P, 512], BF, tag="ost", bufs=3,
                                      name=f"ost_{j}_{do}")
                    nc.vector.tensor_copy(ost[:], po[:])
                    oeng = (nc.scalar if tail and do % 3 == 2
                            else ld_rot[do % 2])
                    oeng.dma_start(out_ext[do * P:(do + 1) * P, js],
                                   ost[:])

            po_pre = {j: [None] * NCC for j in range(NJ)}
            po_work = []

            # ================= attention =================
            for hp in range(NHP):
                for j in range(NJ):
                    js = slice(j * 512, (j + 1) * 512)
                    acc_a = ps_pv.tile([P, 512], F32, tag="pv")
                    acc_b = ps_pv.tile([P, 512], F32, tag="pv")
                    if hp == NHP - 1 and j == NJ - 1:
                        # prefetch j=3's six early out-projection inputs
                        po_loads(NJ - 1, po_pre[NJ - 1])
                    for i in range(NI):
                        tbk, ik = i // 4, i % 4
                        ks = slice(ik * 128, (ik + 1) * 128)
                        sc = ps_s.tile([P, 1024], F32, tag="sc")
                        # the exp stream lives or dies by these two being
                        # scheduled the moment their PSUM buffer frees; the
                        # boost must stay local (~1 iteration) — offset 96
                        # let attention race ahead of the block-end
                        # normalize/exchange chains and knotted the vector
                        # and gpsimd queues at head-pair boundaries
                        with tc.high_priority(offset=16):
                            nc.tensor.matmul(sc[:, 0:512],
                                             kb[hp][tbk][0:64, ks],
                                             qs[hp][j][0:64, :],
                                             start=True, stop=True)
                            nc.tensor.matmul(sc[:, 512:1024],
                                             kb[hp][tbk][64:128, ks],
                                             qs[hp][j][64:128, :],
                                             start=True, stop=True)
                        ex = stpool.tile([P, 1024], BF, tag="ex", bufs=8)
                        nc.scalar.activation(ex[:], sc[:], EXP, scale=0.125)
                        # first block: K(0,tb) and V chunk i are produced
                        # inline as their tb-blocks land from DRAM; Q(0,1)
                        # lands in the later iterations, in time for j1.
                        if hp == 0 and j == 0:
                            if i in k0:
                                run_all(k0[i][0])
                            if i >= 4:
                                run_all(v_gen(i))
                            if i >= 10:
                                pop_work(q01, 2)
                        nc.tensor.matmul(acc_a[0:65, :],
                                         vt[i][:, 2 * hp, :],
                                         ex[:, 0:512],
                                         start=(i == 0), stop=(i == NI - 1))
                        nc.tensor.matmul(acc_b[0:65, :],
                                         vt[i][:, 2 * hp + 1, :],
                                         ex[:, 512:1024],
                                         start=(i == 0), stop=(i == NI - 1))
                        if not (hp == 0 and j == 0):
                            pop_work(deferred, 2 if j == 3 else 1)
                        # drip the out projection into head-pair 3
                        if hp == NHP - 1 and j >= 1:
                            pop_work(po_work, 2)
                    # evict BOTH accumulators first (frees the PSUM ring for
                    # the next j-block before the slow reciprocals run),
                    # then normalize: ao[:, js] = acc[0:64] / acc[64]
                    pvsts, recs, bcs = [], [], []
                    for half, acc in ((0, acc_a), (1, acc_b)):
                        pvst = stpool.tile([P, 512], F32, tag="pvst", bufs=4,
                                           name=f"pvst_{hp}_{j}_{half}")
                        nc.vector.tensor_copy(pvst[0:65, :], acc[0:65, :])
                        pvsts.append(pvst)
                    for half in (0, 1):
                        # reciprocal_approx_fast is only correct with
                        # partition-0 operands (verified on hw), so hop the
                        # denominator row down first
                        den = stpool.tile([P, 512], F32, tag="den", bufs=2)
                        nc.vector.tensor_copy(den[0:1, :],
                                              pvsts[half][64:65, :])
                        rec = stpool.tile([P, 512], F32, tag="rec", bufs=2)
                        nc.vector.reciprocal_approx_fast(rec[0:1, :],
                                                         den[0:1, :])
                        recs.append(rec)
                        bc = stpool.tile([P, 512], F32, tag="bc", bufs=2)
                        nc.gpsimd.partition_broadcast(bc[0:64, :],
                                                      rec[0:1, :],
                                                      channels=64)
                        bcs.append(bc)
                    for half in (0, 1):
                        nc.vector.tensor_tensor(
                            ao[hp][half * 64:(half + 1) * 64, js],
                            pvsts[half][0:64, :], bcs[half][0:64, :], op=MUL)
                    # exchanges: head-pairs 0-2 once per hp (overlapped with
                    # the next head-pair); head-pair 3 per j-block so the
                    # output projection can start before attention ends.
                    if hp == NHP - 1:
                        # final block's exchange input skips sync's ring
                        # backlog; scalar is idle after the last exp issues
                        aeng = nc.scalar if j == NJ - 1 else nc.sync
                        aeng.dma_start(agi3[j][:], ao[hp][:, js])
                        nc.gpsimd.collective_compute(
                            "AllGather", mybir.AluOpType.bypass,
                            replica_groups=groups,
                            ins=[agi3[j].opt()], outs=[ago3[j].opt()])
                        if j < NJ - 1:
                            po_loads(j, po_pre[j])
                            po_work.insert(0, po_gen(j, po_pre[j]))
                if hp < NHP - 1:
                    nc.sync.dma_start(agi[hp][:], ao[hp][:])
                    nc.gpsimd.collective_compute(
                        "AllGather", mybir.AluOpType.bypass,
                        replica_groups=groups,
                        ins=[agi[hp].opt()], outs=[ago[hp].opt()])

            # ===== output projection tail =====
            for gen in reversed(po_work):
                run_all(gen)
            run_all(po_gen(NJ - 1, po_pre[NJ - 1]))

    nc.compile()
    return nc


def make_in_maps(q_tokens, kv_tokens, Wq, Wk, Wv, Wo):
    q_tokens = np.asarray(q_tokens, np.float32)
    kv_tokens = np.asarray(kv_tokens, np.float32)
    Wq = np.asarray(Wq, np.float32)
    Wk = np.asarray(Wk, np.float32)
    Wv = np.asarray(Wv, np.float32)
    Wo = np.asarray(Wo, np.float32)

    def chunked(w):
        # [in, out] -> [128, n_chunks*out]: contiguous per-partition image
        # of the SBUF-resident [P, n, out] weight tiles
        n = w.shape[0] // P
        return np.ascontiguousarray(
            w.reshape(n, P, w.shape[1]).transpose(1, 0, 2).reshape(P, -1)
        ).astype(_BF16)

    def chunked_hp(w):
        # [in, 512] -> [128, hp, chunk, 128] image so one contiguous
        # transfer delivers one head-pair's slice of every input chunk
        n = w.shape[0] // P
        t = w.reshape(n, P, NHP, P)           # [chunk, p, hp, d]
        return np.ascontiguousarray(
            t.transpose(1, 2, 0, 3).reshape(P, -1)
        ).astype(_BF16)

    in_maps = []
    for c in range(N_CORES):
        b, hg = c // 2, c % 2
        sl = slice(hg * DQ, (hg + 1) * DQ)
        osl = slice(hg * DO, (hg + 1) * DO)
        in_maps.append({
            "xqT": np.ascontiguousarray(q_tokens[b].T).astype(_BF16),
            "xkvT": np.ascontiguousarray(kv_tokens[b].T).astype(_BF16),
            "wqT": chunked_hp(Wq[sl, :].T),
            "wkT": chunked_hp(Wk[sl, :].T),
            "wvT": chunked(Wv[sl, :].T),
            # [dc, do-half] with dc rows in gathered (global head) order
            "woT": chunked(Wo[osl, :].T),
            "onesc": np.ones((P, 8), _BF16),
        })
    return in_maps


def kernel(q_tokens, kv_tokens, Wq, Wk, Wv, Wo):
    global _compiled
    if _compiled is None:
        _compiled = _build()
    nc = _compiled

    in_maps = make_in_maps(q_tokens, kv_tokens, Wq, Wk, Wv, Wo)
    res = bass_utils.run_bass_kernel_spmd(nc, in_maps,
                                          core_ids=list(range(N_CORES)))
    B = 4
    out = np.empty((B, TQ, 2 * DO), np.float32)
    for c in range(N_CORES):
        b, hg = c // 2, c % 2
        out[b, :, hg * DO:(hg + 1) * DO] = \
            np.asarray(res.results[c]["out"], np.float32).T
    return out


# revision 20
# speedup vs baseline: 1.1819x; 1.1819x over previous
"""Cross-attention kernel for 8 Trainium2 NeuronCores (SPMD).

Problem: B=4, T_q=T_kv=2048, Q_DIM=1024, KV_DIM=768, H=16, DK=64, fp32.
  q = q_tokens @ Wq.T ; k = kv_tokens @ Wk.T ; v = kv_tokens @ Wv.T
  out = softmax(q k^T / sqrt(DK)) v @ Wo.T

Sharding (8 cores): core c handles batch b=c//2 and head-group hg=c%2
(8 heads, 512 of the 1024 q-dims).  After attention, the pair (2b, 2b+1)
AllGathers the per-head-group attention outputs, then each core runs the
output projection against ITS half of the Wo columns — core c returns
out[b, :, (c%2)*512:(c%2+1)*512] transposed.  The rank-dependent
output-channel split lives entirely in the host-side Wo slice, so the
device program is identical on all cores.

Engine balance (measured): the window is paced by the scalar engine
(256 exps x 1113ns = 285us busy; FD=1024 is the PSUM-bank max, fp32
matmul output is mandatory on TRN2) with the PE in near-lockstep
(QK pair 385ns concurrent via auto row-groups + 2 PV x 215ns + ~1.3
projection-MM drip capacity per iteration).  This version organizes
everything around keeping that exp stream dense:
  - wq/wk are loaded PER HEAD-PAIR (hp-major host layout), so the
    first-exp gate is 2.9MB not 3.5MB, round-robined over all five
    engine DMA queues; V for the first 4 kv-chunks is produced inside
    the DMA-wait window before the first exp.
  - QK score matmuls run under tc.high_priority so the tile scheduler
    never parks them behind dripped projection matmuls (this was worth
    ~160ns/iteration of exp stall in every pop-heavy block).
  - Remainder DMA order matches need-by: xkv-tb1, xkv-tb23, xq-tb1,
    wk-hp123, wq-hp123, xq-tb23, wo.
  - Output projection po(j) accumulates its hp0-2 chunks first (DMA'd
    from the early AllGathers, prefetched), so only the last 2 of 8
    matmuls per chain gate on head-pair 3's per-j exchange; j=3's
    early chunks are prefetched at block start and the final out
    stores fan out over three DMA queues.

Measured state: 403,058ns = ~31us head (first exp; scalar's DMA ring
must drain its input triggers first — a trigger on a full ring blocks
the queue at transfer-completion pace, so scalar only gets triggers
through xkv-tb1) + ~347us window (ACT 288us busy; ~22us j0 idle is
V-production PE capacity + remainder-DMA pacing, ~12us hp3 is po
overload, ~5us at head-pair boundaries from the normalize/exchange
chain) + ~25us tail-to-drain.

Measured dead ends, do not retry: exp FD=2048 (PSUM bank budget: 4
score banks + 2 PV + 2 projection banks is exactly 8; bf16 matmul
output is TRN3-only), PE warm-up matmuls, ACT-table preload, batched
3D strided input DMAs (device-fatal descriptor fault),
reciprocal_approx_fast off partition 0 (silently wrong),
high_priority offset 96 (knots vector/gpsimd block-end chains, 15us
boundary stalls), ex ring < 8 bufs (-65us!), po stores on gpsimd,
po(3) aog prefetch hoisted to j1 with aog bufs 24 (SBUF + pool-ring
conflicts; regressed to 471us with ex=6).  Run-to-run noise is
+-10-20us from chip power states.
"""

import numpy as np

import concourse.bacc as bacc
import concourse.mybir as mybir
import concourse.tile as tile
from concourse import bass_utils

try:
    import ml_dtypes
    _BF16 = ml_dtypes.bfloat16
except ImportError:  # pragma: no cover
    _BF16 = mybir.dt.np(mybir.dt.bfloat16)

N_CORES = 8
P = 128
TQ = 2048
TKV = 2048
CQ = 1024     # q_tokens channels
CKV = 768     # kv_tokens channels
DQ = 512      # per-core head-group q dims (8 heads x 64)
DO = 512      # per-core output channels (half of 1024)
NJ = 4        # 512-wide tq j-blocks (== projection t-blocks)
NI = TKV // P  # 16 kv chunks
NHP = DQ // P  # 4 head-pairs
CQ_CH = CQ // P   # 8
CKV_CH = CKV // P  # 6
NCC = 2 * NHP     # 8 dc chunks in the gathered attention output

F32 = mybir.dt.float32
BF = mybir.dt.bfloat16
EXP = mybir.ActivationFunctionType.Exp
MUL = mybir.AluOpType.mult

_compiled = None


def _build():
    nc = bacc.Bacc("TRN2", target_bir_lowering=False, debug=False,
                   num_devices=N_CORES)

    xqT = nc.dram_tensor("xqT", [CQ, TQ], BF, kind="ExternalInput")
    xkvT = nc.dram_tensor("xkvT", [CKV, TKV], BF, kind="ExternalInput")
    # wq/wk come hp-major from the host: [128, hp, chunk, 128] flattened,
    # so one contiguous transfer delivers exactly one head-pair's slice.
    wqT = nc.dram_tensor("wqT", [P, NHP * CQ_CH * P], BF,
                         kind="ExternalInput")
    wkT = nc.dram_tensor("wkT", [P, NHP * CKV_CH * P], BF,
                         kind="ExternalInput")
    wvT = nc.dram_tensor("wvT", [P, CKV_CH * DQ], BF, kind="ExternalInput")
    # full-dc Wo slice for this core's output-channel half, dc rows in
    # gathered order (head-group 0 rows then head-group 1 rows)
    woT = nc.dram_tensor("woT", [P, NCC * DO], BF, kind="ExternalInput")
    onesc = nc.dram_tensor("onesc", [P, 8], BF, kind="ExternalInput")
    out_ext = nc.dram_tensor("out", [DO, TQ], BF, kind="ExternalOutput")

    groups = [[2 * b, 2 * b + 1] for b in range(N_CORES // 2)]

    with tile.TileContext(nc) as tc:
        with (
            tc.tile_pool(name="weights", bufs=1) as wpool,
            tc.tile_pool(name="xres", bufs=1) as xpool,
            tc.tile_pool(name="kqv", bufs=1) as kpool,
            tc.tile_pool(name="stage", bufs=1) as stpool,
            tc.tile_pool(name="dram", bufs=1, space="DRAM") as dpool,
            tc.tile_pool(name="psum_s", bufs=2, space="PSUM") as ps_s,
            tc.tile_pool(name="psum_pv", bufs=2, space="PSUM") as ps_pv,
            tc.tile_pool(name="psum_pj", bufs=2, space="PSUM") as ps_pj,
        ):
            # ---- resident weights + token inputs (bf16) ----
            wk_sb = wpool.tile([P, NHP, CKV_CH, P], BF, tag="wk")
            wq_sb = wpool.tile([P, NHP, CQ_CH, P], BF, tag="wq")
            wv_sb = wpool.tile([P, CKV_CH, DQ], BF, tag="wv")
            wo_sb = wpool.tile([P, NCC, DO], BF, tag="wo")
            ones_sb = wpool.tile([P, 8, 1], BF, tag="ones")
            xkv_sb = [xpool.tile([P, TKV], BF, tag="xkv", bufs=CKV_CH,
                                 name=f"xkv{c}") for c in range(CKV_CH)]
            xq_sb = [xpool.tile([P, TQ], BF, tag="xq", bufs=CQ_CH,
                                name=f"xq{c}") for c in range(CQ_CH)]

            # All input transfers issued up front on the three DMA-capable
            # queues (SP/gpsimd/ACT — triggers are ~650ns each and all fit
            # in the scalar queue's pre-first-exp idle), in need-by order;
            # per-queue transfers execute in trigger order, so issue order
            # IS arrival order.  Prefix gating the first exp: wk-hp0,
            # xkv-tb0, wv (feeds prelude V), wq-hp0, xq-tb0 = 2.9MB.  Then
            # xkv-tb1 (K(0,1)+V mid-j0), xkv-tb23, xq-tb1 (Q(0,1) by j1),
            # wk/wq hp1-3, xq-tb23 (Q(0,2)+ by j2), wo (head-pair 3).
            q3 = [nc.sync, nc.gpsimd, nc.scalar]
            xfers = [(wk_sb[:, 0], wkT.ap()[:, 0:CKV_CH * P])]
            for c in range(CKV_CH):
                xfers.append((xkv_sb[c][:, 0:512],
                              xkvT.ap()[c * P:(c + 1) * P, 0:512]))
            for c in range(CKV_CH):
                xfers.append((wv_sb[:, c, :],
                              wvT.ap()[:, c * DQ:(c + 1) * DQ]))
            xfers.append((wq_sb[:, 0], wqT.ap()[:, 0:CQ_CH * P]))
            for c in range(CQ_CH):
                xfers.append((xq_sb[c][:, 0:512],
                              xqT.ap()[c * P:(c + 1) * P, 0:512]))
            for c in range(CKV_CH):
                xfers.append((xkv_sb[c][:, 512:1024],
                              xkvT.ap()[c * P:(c + 1) * P, 512:1024]))
            for c in range(CKV_CH):
                xfers.append((xkv_sb[c][:, 1024:TKV],
                              xkvT.ap()[c * P:(c + 1) * P, 1024:TKV]))
            for c in range(CQ_CH):
                xfers.append((xq_sb[c][:, 512:1024],
                              xqT.ap()[c * P:(c + 1) * P, 512:1024]))
            for hp in range(1, NHP):
                xfers.append((wk_sb[:, hp],
                              wkT.ap()[:, hp * CKV_CH * P:(hp + 1) * CKV_CH * P]))
                xfers.append((wq_sb[:, hp],
                              wqT.ap()[:, hp * CQ_CH * P:(hp + 1) * CQ_CH * P]))
            for c in range(CQ_CH):
                xfers.append((xq_sb[c][:, 1024:TQ],
                              xqT.ap()[c * P:(c + 1) * P, 1024:TQ]))
            for cc in range(NCC):
                xfers.append((wo_sb[:, cc, :],
                              woT.ap()[:, cc * DO:(cc + 1) * DO]))
            # ones first (2KB — gates the vt ones-columns for the first PV)
            nc.gpsimd.dma_start(ones_sb[:],
                                onesc.ap().rearrange("p (n o) -> p n o", o=1))
            # A DMA trigger BLOCKS its engine queue while the hardware ring
            # is full, pacing at transfer-completion rate — so the scalar
            # queue only gets triggers that drain before the first exp
            # (~5 x 128-192KB); everything later goes to sync/gpsimd.
            # (three queues through xkv-tb1: scalar's ring drains those
            # triggers before the first exp, and j0's inline K/V work
            # otherwise starves on the 2-queue remainder stream)
            for n, (dst, src) in enumerate(xfers):
                if n < 28:
                    q3[n % 3].dma_start(dst, src)
                else:
                    q3[n % 2].dma_start(dst, src)

            # ---- SBUF-resident K/Q/V (written by projection evictions) ----
            kb = [[kpool.tile([P, 512], BF, tag="kb", bufs=NHP * NJ,
                              name=f"kb{hp}_{tb}") for tb in range(NJ)]
                  for hp in range(NHP)]
            qs = [[kpool.tile([P, 512], BF, tag="qs", bufs=NHP * NJ,
                              name=f"qs{hp}_{tb}") for tb in range(NJ)]
                  for hp in range(NHP)]
            # vt[tc]: [128 tkv-chunk, 8 heads, 64+1] (ones col -> denominator)
            vt = [kpool.tile([P, 8, 65], BF, tag="vt", bufs=NI,
                             name=f"vt{tc}") for tc in range(NI)]
            # normalized attention output per head-pair (exchanged via CC)
            ao = [kpool.tile([P, TQ], BF, tag="ao", bufs=NHP,
                             name=f"ao{hp}") for hp in range(NHP)]

            # ---- internal DRAM for collectives ----
            agi = [dpool.tile([P, TQ], BF, tag=f"agi{h}", name=f"agi{h}")
                   for h in range(NHP - 1)]
            ago = [dpool.tile([2, P, TQ], BF, tag=f"ago{h}", name=f"ago{h}")
                   for h in range(NHP - 1)]
            agi3 = [dpool.tile([P, 512], BF, tag=f"agi3_{j}", name=f"agi3_{j}")
                    for j in range(NJ)]
            ago3 = [dpool.tile([2, P, 512], BF, tag=f"ago3_{j}",
                               name=f"ago3_{j}") for j in range(NJ)]

            # ============ projection work units (one yield per MM) ========
            def k_gen(hp, tb):
                pk = ps_pj.tile([P, 512], F32, tag="pj", name=f"pk_{hp}_{tb}")
                for c in range(CKV_CH):
                    nc.tensor.matmul(pk[:], wk_sb[:, hp, c, :],
                                     xkv_sb[c][:, tb * 512:(tb + 1) * 512],
                                     start=(c == 0), stop=(c == CKV_CH - 1))
                    if c == CKV_CH - 1:
                        nc.vector.tensor_copy(kb[hp][tb][:], pk[:])
                    yield

            def q_gen(hp, tb):
                pq = ps_pj.tile([P, 512], F32, tag="pj", name=f"pq_{hp}_{tb}")
                for c in range(CQ_CH):
                    nc.tensor.matmul(pq[:], wq_sb[:, hp, c, :],
                                     xq_sb[c][:, tb * 512:(tb + 1) * 512],
                                     start=(c == 0), stop=(c == CQ_CH - 1))
                    if c == CQ_CH - 1:
                        nc.vector.tensor_copy(qs[hp][tb][:], pq[:])
                    yield

            def v_gen(tc_i):
                pv = ps_pj.tile([P, 512], F32, tag="pj", name=f"pv_{tc_i}")
                for c in range(CKV_CH):
                    nc.tensor.matmul(
                        pv[:],
                        xkv_sb[c][:, tc_i * P:(tc_i + 1) * P],
                        wv_sb[:, c, :],
                        start=(c == 0), stop=(c == CKV_CH - 1))
                    if c == CKV_CH - 1:
                        nc.vector.tensor_copy(
                            vt[tc_i][:, :, 0:64],
                            pv[:].rearrange("p (h d) -> p h d", d=64))
                        nc.vector.tensor_copy(vt[tc_i][:, :, 64:65],
                                              ones_sb[:])
                    yield

            def run_all(gen):
                for _ in gen:
                    pass

            # deferred projection work, drip-fed into the attention loop.
            # Order respects need-by times: Q(hp,tb) before block (hp,tb)
            # starts, K(hp) fully before head-pair hp starts.
            deferred = [
                q_gen(0, 2),
                k_gen(1, 0), k_gen(1, 1),
                q_gen(0, 3),
                k_gen(1, 2), k_gen(1, 3),
                q_gen(1, 0), q_gen(1, 1),
                k_gen(2, 0), k_gen(2, 1), k_gen(2, 2), k_gen(2, 3),
                q_gen(1, 2), q_gen(1, 3),
                q_gen(2, 0),
                k_gen(3, 0), k_gen(3, 1),
                q_gen(2, 1), q_gen(2, 2),
                k_gen(3, 2), k_gen(3, 3),
                q_gen(2, 3),
                q_gen(3, 0), q_gen(3, 1), q_gen(3, 2), q_gen(3, 3),
            ]
            deferred.reverse()

            def pop_work(queue, n):
                while n > 0 and queue:
                    gen = queue[-1]
                    try:
                        next(gen)
                        n -= 1
                    except StopIteration:
                        queue.pop()

            # ================= prelude =================
            # K(0,0) + V(0..3) + Q(0,0).  V production fills the PE-idle
            # DMA-wait window (wv+xkv-tb0 arrive well before wq/xq-tb0);
            # the scheduler lets Q00 overtake any V matmul still waiting
            # on data.  K(0,1..3) and V(4..15) are produced inside j0's
            # iterations as their chunks land; Q(0,1) late in j0.
            run_all(k_gen(0, 0))
            for t in range(4):
                run_all(v_gen(t))
            run_all(q_gen(0, 0))
            k0 = {2: [k_gen(0, 1)], 6: [k_gen(0, 2)], 10: [k_gen(0, 3)]}
            q01 = [q_gen(0, 1)]

            # ============== out-projection work units ==============
            # Chunk order per chain: head-pairs 0-2 first (gathered long
            # ago, DMA'd with no wait), head-pair 3's pair last so only 2
            # of 8 matmuls gate on the final per-j exchange.
            po_order = [(g, hpx) for hpx in range(NHP) for g in range(2)]
            # scalar stays exp-only until the tail, and gpsimd carries the
            # collectives + partition broadcasts — po traffic goes on sync
            # (j=3's stores can use scalar once the exp stream has drained)
            ld_rot = [nc.sync, nc.sync]

            def po_loads(j, pre):
                js = slice(j * 512, (j + 1) * 512)
                for n, (g, hpx) in enumerate(po_order):
                    if hpx == NHP - 1:
                        continue
                    aog = stpool.tile([P, 512], BF, tag="aog", bufs=16,
                                      name=f"aog_{j}_{g}_{hpx}")
                    ld_rot[n % 2].dma_start(aog[:], ago[hpx][g, :, js])
                    pre[n] = aog

            def po_gen(j, pre):
                tail = j == NJ - 1
                for n, (g, hpx) in enumerate(po_order):
                    if hpx == NHP - 1:
                        aog = stpool.tile([P, 512], BF, tag="aog", bufs=16,
                                          name=f"aog_{j}_{g}_{hpx}")
                        eng = nc.scalar if tail else ld_rot[n % 2]
                        eng.dma_start(aog[:], ago3[j][g, :, :])
                        pre[n] = aog
                js = slice(j * 512, (j + 1) * 512)
                for do in range(DO // P):
                    po = ps_pj.tile([P, 512], F32, tag="pj",
                                    name=f"po_{j}_{do}")
                    for n, (g, hpx) in enumerate(po_order):
                        cc = g * NHP + hpx
                        nc.tensor.matmul(
                            po[:], wo_sb[:, cc, do * P:(do + 1) * P],
                            pre[n][:],
                            start=(n == 0), stop=(n == NCC - 1))
                        yield
                    ost = stpool.tile([P, 512], BF, tag="ost", bufs=3,
                                      name=f"ost_{j}_{do}")
                    nc.vector.tensor_copy(ost[:], po[:])
                    oeng = (nc.scalar if tail and do % 3 == 2
                            else ld_rot[do % 2])
                    oeng.dma_start(out_ext[do * P:(do + 1) * P, js],
                                   ost[:])

            po_pre = {j: [None] * NCC for j in range(NJ)}
            po_work = []

            # ================= attention =================
            for hp in range(NHP):
                for j in range(NJ):
                    js = slice(j * 512, (j + 1) * 512)
                    acc_a = ps_pv.tile([P, 512], F32, tag="pv")
                    acc_b = ps_pv.tile([P, 512], F32, tag="pv")
                    if hp == NHP - 1 and j == NJ - 1:
                        # prefetch j=3's six early out-projection inputs
                        po_loads(NJ - 1, po_pre[NJ - 1])
                    for i in range(NI):
                        tbk, ik = i // 4, i % 4
                        ks = slice(ik * 128, (ik + 1) * 128)
                        sc = ps_s.tile([P, 1024], F32, tag="sc")
                        # the exp stream lives or dies by these two being
                        # scheduled the moment their PSUM buffer frees; the
                        # boost must stay local (~1 iteration) — offset 96
                        # let attention race ahead of the block-end
                        # normalize/exchange chains and knotted the vector
                        # and gpsimd queues at head-pair boundaries
                        with tc.high_priority(offset=16):
                            nc.tensor.matmul(sc[:, 0:512],
                                             kb[hp][tbk][0:64, ks],
                                             qs[hp][j][0:64, :],
                                             start=True, stop=True)
                            nc.tensor.matmul(sc[:, 512:1024],
                                             kb[hp][tbk][64:128, ks],
                                             qs[hp][j][64:128, :],
                                             start=True, stop=True)
                        ex = stpool.tile([P, 1024], BF, tag="ex", bufs=8)
                        nc.scalar.activation(ex[:], sc[:], EXP, scale=0.125)
                        # first block: K(0,tb) and V chunk i are produced
                        # inline as their tb-blocks land from DRAM; Q(0,1)
                        # lands in the later iterations, in time for j1.
                        if hp == 0 and j == 0:
                            if i in k0:
                                run_all(k0[i][0])
                            if i >= 4:
                                run_all(v_gen(i))
                            if i >= 10:
                                pop_work(q01, 2)
                        nc.tensor.matmul(acc_a[0:65, :],
                                         vt[i][:, 2 * hp, :],
                                         ex[:, 0:512],
                                         start=(i == 0), stop=(i == NI - 1))
                        nc.tensor.matmul(acc_b[0:65, :],
                                         vt[i][:, 2 * hp + 1, :],
                                         ex[:, 512:1024],
                                         start=(i == 0), stop=(i == NI - 1))
                        if not (hp == 0 and j == 0):
                            pop_work(deferred, 2 if j == 3 else 1)
                        # drip the out projection into head-pair 3
                        if hp == NHP - 1 and j >= 1:
                            pop_work(po_work, 2)
                    # evict BOTH accumulators first (frees the PSUM ring for
                    # the next j-block before the slow reciprocals run),
                    # then normalize: ao[:, js] = acc[0:64] / acc[64]
                    pvsts, recs, bcs = [], [], []
                    for half, acc in ((0, acc_a), (1, acc_b)):
                        pvst = stpool.tile([P, 512], F32, tag="pvst", bufs=4,
                                           name=f"pvst_{hp}_{j}_{half}")
                        nc.vector.tensor_copy(pvst[0:65, :], acc[0:65, :])
                        pvsts.append(pvst)
                    for half in (0, 1):
                        # reciprocal_approx_fast is only correct with
                        # partition-0 operands (verified on hw), so hop the
                        # denominator row down first
                        den = stpool.tile([P, 512], F32, tag="den", bufs=2)
                        nc.vector.tensor_copy(den[0:1, :],
                                              pvsts[half][64:65, :])
                        rec = stpool.tile([P, 512], F32, tag="rec", bufs=2)
                        nc.vector.reciprocal_approx_fast(rec[0:1, :],
                                                         den[0:1, :])
                        recs.append(rec)
                        bc = stpool.tile([P, 512], F32, tag="bc", bufs=2)
                        nc.gpsimd.partition_broadcast(bc[0:64, :],
                                                      rec[0:1, :],
                                                      channels=64)
                        bcs.append(bc)
                    for half in (0, 1):
                        nc.vector.tensor_tensor(
                            ao[hp][half * 64:(half + 1) * 64, js],
                            pvsts[half][0:64, :], bcs[half][0:64, :], op=MUL)
                    # exchanges: head-pairs 0-2 once per hp (overlapped with
                    # the next head-pair); head-pair 3 per j-block so the
                    # output projection can start before attention ends.
                    if hp == NHP - 1:
                        # final block's exchange input skips sync's ring
                        # backlog; scalar is idle after the last exp issues
                        aeng = nc.scalar if j == NJ - 1 else nc.sync
                        aeng.dma_start(agi3[j][:], ao[hp][:, js])
                        nc.gpsimd.collective_compute(
                            "AllGather", mybir.AluOpType.bypass,
                            replica_groups=groups,
                            ins=[agi3[j].opt()], outs=[ago3[j].opt()])
                        if j < NJ - 1:
                            po_loads(j, po_pre[j])
                            po_work.insert(0, po_gen(j, po_pre[j]))
                if hp < NHP - 1:
                    nc.sync.dma_start(agi[hp][:], ao[hp][:])
                    nc.gpsimd.collective_compute(
                        "AllGather", mybir.AluOpType.bypass,
                        replica_groups=groups,
                        ins=[agi[hp].opt()], outs=[ago[hp].opt()])

            # ===== output projection tail =====
            for gen in reversed(po_work):
                run_all(gen)
            run_all(po_gen(NJ - 1, po_pre[NJ - 1]))

    nc.compile()
    return nc


def make_in_maps(q_tokens, kv_tokens, Wq, Wk, Wv, Wo):
    q_tokens = np.asarray(q_tokens, np.float32)
    kv_tokens = np.asarray(kv_tokens, np.float32)
    Wq = np.asarray(Wq, np.float32)
    Wk = np.asarray(Wk, np.float32)
    Wv = np.asarray(Wv, np.float32)
    Wo = np.asarray(Wo, np.float32)

    def chunked(w):
        # [in, out] -> [128, n_chunks*out]: contiguous per-partition image
        # of the SBUF-resident [P, n, out] weight tiles
        n = w.shape[0] // P
        return np.ascontiguousarray(
            w.reshape(n, P, w.shape[1]).transpose(1, 0, 2).reshape(P, -1)
        ).astype(_BF16)

    def chunked_hp(w):
        # [in, 512] -> [128, hp, chunk, 128] image so one contiguous
        # transfer delivers one head-pair's slice of every input chunk
        n = w.shape[0] // P
        t = w.reshape(n, P, NHP, P)           # [chunk, p, hp, d]
        return np.ascontiguousarray(
            t.transpose(1, 2, 0, 3).reshape(P, -1)
        ).astype(_BF16)

    in_maps = []
    for c in range(N_CORES):
        b, hg = c // 2, c % 2
        sl = slice(hg * DQ, (hg + 1) * DQ)
        osl = slice(hg * DO, (hg + 1) * DO)
        in_maps.append({
            "xqT": np.ascontiguousarray(q_tokens[b].T).astype(_BF16),
            "xkvT": np.ascontiguousarray(kv_tokens[b].T).astype(_BF16),
            "wqT": chunked_hp(Wq[sl, :].T),
            "wkT": chunked_hp(Wk[sl, :].T),
            "wvT": chunked(Wv[sl, :].T),
            # [dc, do-half] with dc rows in gathered (global head) order
            "woT": chunked(Wo[osl, :].T),
            "onesc": np.ones((P, 8), _BF16),
        })
    return in_maps


def kernel(q_tokens, kv_tokens, Wq, Wk, Wv, Wo):
    global _compiled
    if _compiled is None:
        _compiled = _build()
    nc = _compiled

    in_maps = make_in_maps(q_tokens, kv_tokens, Wq, Wk, Wv, Wo)
    res = bass_utils.run_bass_kernel_spmd(nc, in_maps,
                                          core_ids=list(range(N_CORES)))
    B = 4
    out = np.empty((B, TQ, 2 * DO), np.float32)
    for c in range(N_CORES):
        b, hg = c // 2, c % 2
        out[b, :, hg * DO:(hg + 1) * DO] = \
            np.asarray(res.results[c]["out"], np.float32).T
    return out


# revision 21
# speedup vs baseline: 1.2081x; 1.0222x over previous
"""Cross-attention kernel for 8 Trainium2 NeuronCores (SPMD).

Problem: B=4, T_q=T_kv=2048, Q_DIM=1024, KV_DIM=768, H=16, DK=64, fp32.
  q = q_tokens @ Wq.T ; k = kv_tokens @ Wk.T ; v = kv_tokens @ Wv.T
  out = softmax(q k^T / sqrt(DK)) v @ Wo.T

Sharding (8 cores): core c handles batch b=c//2 and head-group hg=c%2
(8 heads, 512 of the 1024 q-dims).  After attention, the pair (2b, 2b+1)
AllGathers the per-head-group attention outputs, then each core runs the
output projection against ITS half of the Wo columns — core c returns
out[b, :, (c%2)*512:(c%2+1)*512] transposed.  The rank-dependent
output-channel split lives entirely in the host-side Wo slice, so the
device program is identical on all cores.

Engine balance (measured): the window is paced by the scalar engine
(256 exps x 1113ns = 285us busy; FD=1024 is the PSUM-bank max, fp32
matmul output is mandatory on TRN2) with the PE in near-lockstep
(QK pair 385ns concurrent via auto row-groups + 2 PV x 215ns + ~1.3
projection-MM drip capacity per iteration).  This version organizes
everything around keeping that exp stream dense:
  - wq/wk are loaded PER HEAD-PAIR (hp-major host layout), so the
    first-exp gate is 2.9MB not 3.5MB, round-robined over all five
    engine DMA queues; V for the first 4 kv-chunks is produced inside
    the DMA-wait window before the first exp.
  - QK score matmuls run under tc.high_priority so the tile scheduler
    never parks them behind dripped projection matmuls (this was worth
    ~160ns/iteration of exp stall in every pop-heavy block).
  - Remainder DMA order matches need-by: xkv-tb1, xkv-tb23, xq-tb1,
    wk-hp123, wq-hp123, xq-tb23, wo.
  - Output projection po(j) accumulates its hp0-2 chunks first (DMA'd
    from the early AllGathers, prefetched), so only the last 2 of 8
    matmuls per chain gate on head-pair 3's per-j exchange; j=3's
    early chunks are prefetched at block start and the final out
    stores fan out over three DMA queues.

Measured state: 403,058ns = ~31us head (first exp; scalar's DMA ring
must drain its input triggers first — a trigger on a full ring blocks
the queue at transfer-completion pace, so scalar only gets triggers
through xkv-tb1) + ~347us window (ACT 288us busy; ~22us j0 idle is
V-production PE capacity + remainder-DMA pacing, ~12us hp3 is po
overload, ~5us at head-pair boundaries from the normalize/exchange
chain) + ~25us tail-to-drain.

Measured dead ends, do not retry: exp FD=2048 (PSUM bank budget: 4
score banks + 2 PV + 2 projection banks is exactly 8; bf16 matmul
output is TRN3-only), PE warm-up matmuls, ACT-table preload, batched
3D strided input DMAs (device-fatal descriptor fault),
reciprocal_approx_fast off partition 0 (silently wrong),
high_priority offset 96 (knots vector/gpsimd block-end chains, 15us
boundary stalls), ex ring < 8 bufs (-65us!), po stores on gpsimd,
po(3) aog prefetch hoisted to j1 with aog bufs 24 (SBUF + pool-ring
conflicts; regressed to 471us with ex=6).  Run-to-run noise is
+-10-20us from chip power states.
"""

import numpy as np

import concourse.bacc as bacc
import concourse.mybir as mybir
import concourse.tile as tile
from concourse import bass_utils

try:
    import ml_dtypes
    _BF16 = ml_dtypes.bfloat16
except ImportError:  # pragma: no cover
    _BF16 = mybir.dt.np(mybir.dt.bfloat16)

N_CORES = 8
P = 128
TQ = 2048
TKV = 2048
CQ = 1024     # q_tokens channels
CKV = 768     # kv_tokens channels
DQ = 512      # per-core head-group q dims (8 heads x 64)
DO = 512      # per-core output channels (half of 1024)
NJ = 4        # 512-wide tq j-blocks (== projection t-blocks)
NI = TKV // P  # 16 kv chunks
NHP = DQ // P  # 4 head-pairs
CQ_CH = CQ // P   # 8
CKV_CH = CKV // P  # 6
NCC = 2 * NHP     # 8 dc chunks in the gathered attention output

F32 = mybir.dt.float32
BF = mybir.dt.bfloat16
EXP = mybir.ActivationFunctionType.Exp
MUL = mybir.AluOpType.mult

_compiled = None


def _build():
    nc = bacc.Bacc("TRN2", target_bir_lowering=False, debug=False,
                   num_devices=N_CORES)

    xqT = nc.dram_tensor("xqT", [CQ, TQ], BF, kind="ExternalInput")
    xkvT = nc.dram_tensor("xkvT", [CKV, TKV], BF, kind="ExternalInput")
    # wq/wk come hp-major from the host: [128, hp, chunk, 128] flattened,
    # so one contiguous transfer delivers exactly one head-pair's slice.
    wqT = nc.dram_tensor("wqT", [P, NHP * CQ_CH * P], BF,
                         kind="ExternalInput")
    wkT = nc.dram_tensor("wkT", [P, NHP * CKV_CH * P], BF,
                         kind="ExternalInput")
    wvT = nc.dram_tensor("wvT", [P, CKV_CH * DQ], BF, kind="ExternalInput")
    # full-dc Wo slice for this core's output-channel half, dc rows in
    # gathered order (head-group 0 rows then head-group 1 rows)
    woT = nc.dram_tensor("woT", [P, NCC * DO], BF, kind="ExternalInput")
    onesc = nc.dram_tensor("onesc", [P, 8], BF, kind="ExternalInput")
    out_ext = nc.dram_tensor("out", [DO, TQ], BF, kind="ExternalOutput")

    groups = [[2 * b, 2 * b + 1] for b in range(N_CORES // 2)]

    with tile.TileContext(nc) as tc:
        with (
            tc.tile_pool(name="weights", bufs=1) as wpool,
            tc.tile_pool(name="xres", bufs=1) as xpool,
            tc.tile_pool(name="kqv", bufs=1) as kpool,
            tc.tile_pool(name="stage", bufs=1) as stpool,
            tc.tile_pool(name="dram", bufs=1, space="DRAM") as dpool,
            tc.tile_pool(name="psum_s", bufs=2, space="PSUM") as ps_s,
            tc.tile_pool(name="psum_pv", bufs=2, space="PSUM") as ps_pv,
            tc.tile_pool(name="psum_pj", bufs=2, space="PSUM") as ps_pj,
        ):
            # ---- resident weights + token inputs (bf16) ----
            wk_sb = wpool.tile([P, NHP, CKV_CH, P], BF, tag="wk")
            wq_sb = wpool.tile([P, NHP, CQ_CH, P], BF, tag="wq")
            wv_sb = wpool.tile([P, CKV_CH, DQ], BF, tag="wv")
            wo_sb = wpool.tile([P, NCC, DO], BF, tag="wo")
            ones_sb = wpool.tile([P, 8, 1], BF, tag="ones")
            xkv_sb = [xpool.tile([P, TKV], BF, tag="xkv", bufs=CKV_CH,
                                 name=f"xkv{c}") for c in range(CKV_CH)]
            xq_sb = [xpool.tile([P, TQ], BF, tag="xq", bufs=CQ_CH,
                                name=f"xq{c}") for c in range(CQ_CH)]

            # All input transfers issued up front on the three DMA-capable
            # queues (SP/gpsimd/ACT — triggers are ~650ns each and all fit
            # in the scalar queue's pre-first-exp idle), in need-by order;
            # per-queue transfers execute in trigger order, so issue order
            # IS arrival order.  Prefix gating the first exp: wk-hp0,
            # xkv-tb0, wv (feeds prelude V), wq-hp0, xq-tb0 = 2.9MB.  Then
            # xkv-tb1 (K(0,1)+V mid-j0), xkv-tb23, xq-tb1 (Q(0,1) by j1),
            # wk/wq hp1-3, xq-tb23 (Q(0,2)+ by j2), wo (head-pair 3).
            q3 = [nc.sync, nc.gpsimd, nc.scalar]
            xfers = [(wk_sb[:, 0], wkT.ap()[:, 0:CKV_CH * P])]
            for c in range(CKV_CH):
                xfers.append((xkv_sb[c][:, 0:512],
                              xkvT.ap()[c * P:(c + 1) * P, 0:512]))
            for c in range(CKV_CH):
                xfers.append((wv_sb[:, c, :],
                              wvT.ap()[:, c * DQ:(c + 1) * DQ]))
            xfers.append((wq_sb[:, 0], wqT.ap()[:, 0:CQ_CH * P]))
            for c in range(CQ_CH):
                xfers.append((xq_sb[c][:, 0:512],
                              xqT.ap()[c * P:(c + 1) * P, 0:512]))
            for c in range(CKV_CH):
                xfers.append((xkv_sb[c][:, 512:1024],
                              xkvT.ap()[c * P:(c + 1) * P, 512:1024]))
            for c in range(CKV_CH):
                xfers.append((xkv_sb[c][:, 1024:TKV],
                              xkvT.ap()[c * P:(c + 1) * P, 1024:TKV]))
            for c in range(CQ_CH):
                xfers.append((xq_sb[c][:, 512:1024],
                              xqT.ap()[c * P:(c + 1) * P, 512:1024]))
            for hp in range(1, NHP):
                xfers.append((wk_sb[:, hp],
                              wkT.ap()[:, hp * CKV_CH * P:(hp + 1) * CKV_CH * P]))
                xfers.append((wq_sb[:, hp],
                              wqT.ap()[:, hp * CQ_CH * P:(hp + 1) * CQ_CH * P]))
            for c in range(CQ_CH):
                xfers.append((xq_sb[c][:, 1024:TQ],
                              xqT.ap()[c * P:(c + 1) * P, 1024:TQ]))
            for cc in range(NCC):
                xfers.append((wo_sb[:, cc, :],
                              woT.ap()[:, cc * DO:(cc + 1) * DO]))
            # ones first (2KB — gates the vt ones-columns for the first PV)
            nc.gpsimd.dma_start(ones_sb[:],
                                onesc.ap().rearrange("p (n o) -> p n o", o=1))
            # A DMA trigger BLOCKS its engine queue while the hardware ring
            # is full, pacing at transfer-completion rate — so the scalar
            # queue only gets triggers that drain before the first exp
            # (~5 x 128-192KB); everything later goes to sync/gpsimd.
            # (three queues through xkv-tb1: scalar's ring drains those
            # triggers before the first exp, and j0's inline K/V work
            # otherwise starves on the 2-queue remainder stream)
            for n, (dst, src) in enumerate(xfers):
                if n < 28:
                    q3[n % 3].dma_start(dst, src)
                else:
                    q3[n % 2].dma_start(dst, src)

            # ---- SBUF-resident K/Q/V (written by projection evictions) ----
            kb = [[kpool.tile([P, 512], BF, tag="kb", bufs=NHP * NJ,
                              name=f"kb{hp}_{tb}") for tb in range(NJ)]
                  for hp in range(NHP)]
            qs = [[kpool.tile([P, 512], BF, tag="qs", bufs=NHP * NJ,
                              name=f"qs{hp}_{tb}") for tb in range(NJ)]
                  for hp in range(NHP)]
            # vt[tc]: [128 tkv-chunk, 8 heads, 64+1] (ones col -> denominator)
            vt = [kpool.tile([P, 8, 65], BF, tag="vt", bufs=NI,
                             name=f"vt{tc}") for tc in range(NI)]
            # normalized attention output per head-pair (exchanged via CC)
            ao = [kpool.tile([P, TQ], BF, tag="ao", bufs=NHP,
                             name=f"ao{hp}") for hp in range(NHP)]

            # ---- internal DRAM for collectives ----
            agi = [dpool.tile([P, TQ], BF, tag=f"agi{h}", name=f"agi{h}")
                   for h in range(NHP - 1)]
            ago = [dpool.tile([2, P, TQ], BF, tag=f"ago{h}", name=f"ago{h}")
                   for h in range(NHP - 1)]
            agi3 = [dpool.tile([P, 512], BF, tag=f"agi3_{j}", name=f"agi3_{j}")
                    for j in range(NJ)]
            ago3 = [dpool.tile([2, P, 512], BF, tag=f"ago3_{j}",
                               name=f"ago3_{j}") for j in range(NJ)]

            # ============ projection work units (one yield per MM) ========
            def k_gen(hp, tb):
                pk = ps_pj.tile([P, 512], F32, tag="pj", name=f"pk_{hp}_{tb}")
                for c in range(CKV_CH):
                    nc.tensor.matmul(pk[:], wk_sb[:, hp, c, :],
                                     xkv_sb[c][:, tb * 512:(tb + 1) * 512],
                                     start=(c == 0), stop=(c == CKV_CH - 1))
                    if c == CKV_CH - 1:
                        nc.vector.tensor_copy(kb[hp][tb][:], pk[:])
                    yield

            def q_gen(hp, tb):
                pq = ps_pj.tile([P, 512], F32, tag="pj", name=f"pq_{hp}_{tb}")
                for c in range(CQ_CH):
                    nc.tensor.matmul(pq[:], wq_sb[:, hp, c, :],
                                     xq_sb[c][:, tb * 512:(tb + 1) * 512],
                                     start=(c == 0), stop=(c == CQ_CH - 1))
                    if c == CQ_CH - 1:
                        nc.vector.tensor_copy(qs[hp][tb][:], pq[:])
                    yield

            def v_gen(tc_i):
                pv = ps_pj.tile([P, 512], F32, tag="pj", name=f"pv_{tc_i}")
                for c in range(CKV_CH):
                    nc.tensor.matmul(
                        pv[:],
                        xkv_sb[c][:, tc_i * P:(tc_i + 1) * P],
                        wv_sb[:, c, :],
                        start=(c == 0), stop=(c == CKV_CH - 1))
                    if c == CKV_CH - 1:
                        nc.vector.tensor_copy(
                            vt[tc_i][:, :, 0:64],
                            pv[:].rearrange("p (h d) -> p h d", d=64))
                        nc.vector.tensor_copy(vt[tc_i][:, :, 64:65],
                                              ones_sb[:])
                    yield

            def run_all(gen):
                for _ in gen:
                    pass

            # deferred projection work, drip-fed into the attention loop.
            # Order respects need-by times: Q(hp,tb) before block (hp,tb)
            # starts, K(hp) fully before head-pair hp starts.
            deferred = [
                q_gen(0, 2),
                k_gen(1, 0), k_gen(1, 1),
                q_gen(0, 3),
                k_gen(1, 2), k_gen(1, 3),
                q_gen(1, 0), q_gen(1, 1),
                k_gen(2, 0), k_gen(2, 1), k_gen(2, 2), k_gen(2, 3),
                q_gen(1, 2), q_gen(1, 3),
                q_gen(2, 0),
                k_gen(3, 0), k_gen(3, 1),
                q_gen(2, 1), q_gen(2, 2),
                k_gen(3, 2), k_gen(3, 3),
                q_gen(2, 3),
                q_gen(3, 0), q_gen(3, 1), q_gen(3, 2), q_gen(3, 3),
            ]
            deferred.reverse()

            def pop_work(queue, n):
                while n > 0 and queue:
                    gen = queue[-1]
                    try:
                        next(gen)
                        n -= 1
                    except StopIteration:
                        queue.pop()

            # ================= prelude =================
            # K(0,0) + V(0..3) + Q(0,0).  V production fills the PE-idle
            # DMA-wait window (wv+xkv-tb0 arrive well before wq/xq-tb0);
            # the scheduler lets Q00 overtake any V matmul still waiting
            # on data.  K(0,1..3) and V(4..15) are produced inside j0's
            # iterations as their chunks land; Q(0,1) late in j0.
            run_all(k_gen(0, 0))
            for t in range(4):
                run_all(v_gen(t))
            run_all(q_gen(0, 0))
            k0 = {2: [k_gen(0, 1)], 6: [k_gen(0, 2)], 10: [k_gen(0, 3)]}
            q01 = [q_gen(0, 1)]

            # ============== out-projection work units ==============
            # Chunk order per chain: head-pairs 0-2 first (gathered long
            # ago, DMA'd with no wait), head-pair 3's pair last so only 2
            # of 8 matmuls gate on the final per-j exchange.
            po_order = [(g, hpx) for hpx in range(NHP) for g in range(2)]
            # scalar stays exp-only until the tail, and gpsimd carries the
            # collectives + partition broadcasts — po traffic goes on sync
            # (j=3's stores can use scalar once the exp stream has drained)
            ld_rot = [nc.sync, nc.sync]

            def po_loads(j, pre):
                js = slice(j * 512, (j + 1) * 512)
                for n, (g, hpx) in enumerate(po_order):
                    if hpx == NHP - 1:
                        continue
                    aog = stpool.tile([P, 512], BF, tag="aog", bufs=16,
                                      name=f"aog_{j}_{g}_{hpx}")
                    ld_rot[n % 2].dma_start(aog[:], ago[hpx][g, :, js])
                    pre[n] = aog

            def po_gen(j, pre):
                tail = j == NJ - 1
                for n, (g, hpx) in enumerate(po_order):
                    if hpx == NHP - 1:
                        aog = stpool.tile([P, 512], BF, tag="aog", bufs=16,
                                          name=f"aog_{j}_{g}_{hpx}")
                        eng = nc.scalar if tail else ld_rot[n % 2]
                        eng.dma_start(aog[:], ago3[j][g, :, :])
                        pre[n] = aog
                js = slice(j * 512, (j + 1) * 512)
                for do in range(DO // P):
                    po = ps_pj.tile([P, 512], F32, tag="pj",
                                    name=f"po_{j}_{do}")
                    for n, (g, hpx) in enumerate(po_order):
                        cc = g * NHP + hpx
                        # commit late in the tensor FIFO (negative boost):
                        # these wait on ring-paced aog loads, and committed
                        # early they stall QK pairs behind them (measured
                        # 6-8us exp gaps in the j3 drip region)
                        with tc.high_priority(offset=-64):
                            nc.tensor.matmul(
                                po[:], wo_sb[:, cc, do * P:(do + 1) * P],
                                pre[n][:],
                                start=(n == 0), stop=(n == NCC - 1))
                        yield
                    ost = stpool.tile([P, 512], BF, tag="ost", bufs=3,
                                      name=f"ost_{j}_{do}")
                    nc.vector.tensor_copy(ost[:], po[:])
                    oeng = (nc.scalar if tail and do % 3 == 2
                            else ld_rot[do % 2])
                    oeng.dma_start(out_ext[do * P:(do + 1) * P, js],
                                   ost[:])

            po_pre = {j: [None] * NCC for j in range(NJ)}
            po_work = []

            # ================= attention =================
            for hp in range(NHP):
                for j in range(NJ):
                    js = slice(j * 512, (j + 1) * 512)
                    acc_a = ps_pv.tile([P, 512], F32, tag="pv")
                    acc_b = ps_pv.tile([P, 512], F32, tag="pv")
                    if hp == NHP - 1 and j == NJ - 1:
                        # prefetch j=3's six early out-projection inputs
                        po_loads(NJ - 1, po_pre[NJ - 1])
                    for i in range(NI):
                        tbk, ik = i // 4, i % 4
                        ks = slice(ik * 128, (ik + 1) * 128)
                        sc = ps_s.tile([P, 1024], F32, tag="sc")
                        # the exp stream lives or dies by these two being
                        # scheduled the moment their PSUM buffer frees; the
                        # boost must stay local (~1 iteration) — offset 96
                        # let attention race ahead of the block-end
                        # normalize/exchange chains and knotted the vector
                        # and gpsimd queues at head-pair boundaries
                        with tc.high_priority(offset=16):
                            nc.tensor.matmul(sc[:, 0:512],
                                             kb[hp][tbk][0:64, ks],
                                             qs[hp][j][0:64, :],
                                             start=True, stop=True)
                            nc.tensor.matmul(sc[:, 512:1024],
                                             kb[hp][tbk][64:128, ks],
                                             qs[hp][j][64:128, :],
                                             start=True, stop=True)
                        ex = stpool.tile([P, 1024], BF, tag="ex", bufs=8)
                        nc.scalar.activation(ex[:], sc[:], EXP, scale=0.125)
                        # first block: K(0,tb) and V chunk i are produced
                        # inline as their tb-blocks land from DRAM; Q(0,1)
                        # lands in the later iterations, in time for j1.
                        if hp == 0 and j == 0:
                            if i in k0:
                                run_all(k0[i][0])
                            if i >= 4:
                                run_all(v_gen(i))
                            if i >= 10:
                                pop_work(q01, 2)
                        nc.tensor.matmul(acc_a[0:65, :],
                                         vt[i][:, 2 * hp, :],
                                         ex[:, 0:512],
                                         start=(i == 0), stop=(i == NI - 1))
                        nc.tensor.matmul(acc_b[0:65, :],
                                         vt[i][:, 2 * hp + 1, :],
                                         ex[:, 512:1024],
                                         start=(i == 0), stop=(i == NI - 1))
                        if not (hp == 0 and j == 0):
                            pop_work(deferred, 2 if j == 3 else 1)
                        # drip the out projection into head-pair 3
                        if hp == NHP - 1 and j >= 1:
                            pop_work(po_work, 2)
                    # evict BOTH accumulators first (frees the PSUM ring for
                    # the next j-block before the slow reciprocals run),
                    # then normalize: ao[:, js] = acc[0:64] / acc[64]
                    pvsts, recs, bcs = [], [], []
                    for half, acc in ((0, acc_a), (1, acc_b)):
                        pvst = stpool.tile([P, 512], F32, tag="pvst", bufs=4,
                                           name=f"pvst_{hp}_{j}_{half}")
                        nc.vector.tensor_copy(pvst[0:65, :], acc[0:65, :])
                        pvsts.append(pvst)
                    for half in (0, 1):
                        # reciprocal_approx_fast is only correct with
                        # partition-0 operands (verified on hw), so hop the
                        # denominator row down first
                        den = stpool.tile([P, 512], F32, tag="den", bufs=2)
                        nc.vector.tensor_copy(den[0:1, :],
                                              pvsts[half][64:65, :])
                        rec = stpool.tile([P, 512], F32, tag="rec", bufs=2)
                        nc.vector.reciprocal_approx_fast(rec[0:1, :],
                                                         den[0:1, :])
                        recs.append(rec)
                        bc = stpool.tile([P, 512], F32, tag="bc", bufs=2)
                        nc.gpsimd.partition_broadcast(bc[0:64, :],
                                                      rec[0:1, :],
                                                      channels=64)
                        bcs.append(bc)
                    for half in (0, 1):
                        nc.vector.tensor_tensor(
                            ao[hp][half * 64:(half + 1) * 64, js],
                            pvsts[half][0:64, :], bcs[half][0:64, :], op=MUL)
                    # exchanges: head-pairs 0-2 once per hp (overlapped with
                    # the next head-pair); head-pair 3 per j-block so the
                    # output projection can start before attention ends.
                    if hp == NHP - 1:
                        # final block's exchange input skips sync's ring
                        # backlog; scalar is idle after the last exp issues
                        aeng = nc.scalar if j == NJ - 1 else nc.sync
                        aeng.dma_start(agi3[j][:], ao[hp][:, js])
                        nc.gpsimd.collective_compute(
                            "AllGather", mybir.AluOpType.bypass,
                            replica_groups=groups,
                            ins=[agi3[j].opt()], outs=[ago3[j].opt()])
                        if j < NJ - 1:
                            po_loads(j, po_pre[j])
                            po_work.insert(0, po_gen(j, po_pre[j]))
                if hp < NHP - 1:
                    nc.sync.dma_start(agi[hp][:], ao[hp][:])
                    nc.gpsimd.collective_compute(
                        "AllGather", mybir.AluOpType.bypass,
                        replica_groups=groups,
                        ins=[agi[hp].opt()], outs=[ago[hp].opt()])

            # ===== output projection tail =====
            for gen in reversed(po_work):
                run_all(gen)
            run_all(po_gen(NJ - 1, po_pre[NJ - 1]))

    nc.compile()
    return nc


def make_in_maps(q_tokens, kv_tokens, Wq, Wk, Wv, Wo):
    q_tokens = np.asarray(q_tokens, np.float32)
    kv_tokens = np.asarray(kv_tokens, np.float32)
    Wq = np.asarray(Wq, np.float32)
    Wk = np.asarray(Wk, np.float32)
    Wv = np.asarray(Wv, np.float32)
    Wo = np.asarray(Wo, np.float32)

    def chunked(w):
        # [in, out] -> [128, n_chunks*out]: contiguous per-partition image
        # of the SBUF-resident [P, n, out] weight tiles
        n = w.shape[0] // P
        return np.ascontiguousarray(
            w.reshape(n, P, w.shape[1]).transpose(1, 0, 2).reshape(P, -1)
        ).astype(_BF16)

    def chunked_hp(w):
        # [in, 512] -> [128, hp, chunk, 128] image so one contiguous
        # transfer delivers one head-pair's slice of every input chunk
        n = w.shape[0] // P
        t = w.reshape(n, P, NHP, P)           # [chunk, p, hp, d]
        return np.ascontiguousarray(
            t.transpose(1, 2, 0, 3).reshape(P, -1)
        ).astype(_BF16)

    in_maps = []
    for c in range(N_CORES):
        b, hg = c // 2, c % 2
        sl = slice(hg * DQ, (hg + 1) * DQ)
        osl = slice(hg * DO, (hg + 1) * DO)
        in_maps.append({
            "xqT": np.ascontiguousarray(q_tokens[b].T).astype(_BF16),
            "xkvT": np.ascontiguousarray(kv_tokens[b].T).astype(_BF16),
            "wqT": chunked_hp(Wq[sl, :].T),
            "wkT": chunked_hp(Wk[sl, :].T),
            "wvT": chunked(Wv[sl, :].T),
            # [dc, do-half] with dc rows in gathered (global head) order
            "woT": chunked(Wo[osl, :].T),
            "onesc": np.ones((P, 8), _BF16),
        })
    return in_maps


def kernel(q_tokens, kv_tokens, Wq, Wk, Wv, Wo):
    global _compiled
    if _compiled is None:
        _compiled = _build()
    nc = _compiled

    in_maps = make_in_maps(q_tokens, kv_tokens, Wq, Wk, Wv, Wo)
    res = bass_utils.run_bass_kernel_spmd(nc, in_maps,
                                          core_ids=list(range(N_CORES)))
    B = 4
    out = np.empty((B, TQ, 2 * DO), np.float32)
    for c in range(N_CORES):
        b, hg = c // 2, c % 2
        out[b, :, hg * DO:(hg + 1) * DO] = \
            np.asarray(res.results[c]["out"], np.float32).T
    return out


# revision 22
# speedup vs baseline: 1.2398x; 1.0262x over previous
"""Cross-attention kernel for 8 Trainium2 NeuronCores (SPMD).

Problem: B=4, T_q=T_kv=2048, Q_DIM=1024, KV_DIM=768, H=16, DK=64, fp32.
  q = q_tokens @ Wq.T ; k = kv_tokens @ Wk.T ; v = kv_tokens @ Wv.T
  out = softmax(q k^T / sqrt(DK)) v @ Wo.T

Sharding (8 cores): core c handles batch b=c//2 and head-group hg=c%2
(8 heads, 512 of the 1024 q-dims).  After attention, the pair (2b, 2b+1)
AllGathers the per-head-group attention outputs, then each core runs the
output projection against ITS half of the Wo columns — core c returns
out[b, :, (c%2)*512:(c%2+1)*512] transposed.  The rank-dependent
output-channel split lives entirely in the host-side Wo slice, so the
device program is identical on all cores.

Engine balance (measured): the window is paced by the scalar engine
(256 exps x 1113ns = 285us busy; FD=1024 is the PSUM-bank max, fp32
matmul output is mandatory on TRN2) with the PE in near-lockstep
(QK pair 385ns concurrent via auto row-groups + 2 PV x 215ns + ~1.3
projection-MM drip capacity per iteration).  This version organizes
everything around keeping that exp stream dense:
  - wq/wk are loaded PER HEAD-PAIR (hp-major host layout), so the
    first-exp gate is 2.9MB not 3.5MB, round-robined over all five
    engine DMA queues; V for the first 4 kv-chunks is produced inside
    the DMA-wait window before the first exp.
  - QK score matmuls run under tc.high_priority so the tile scheduler
    never parks them behind dripped projection matmuls (this was worth
    ~160ns/iteration of exp stall in every pop-heavy block).
  - Remainder DMA order matches need-by: xkv-tb1, xkv-tb23, xq-tb1,
    wk-hp123, wq-hp123, xq-tb23, wo.
  - Output projection po(j) accumulates its hp0-2 chunks first (DMA'd
    from the early AllGathers, prefetched), so only the last 2 of 8
    matmuls per chain gate on head-pair 3's per-j exchange; j=3's
    early chunks are prefetched at block start and the final out
    stores fan out over three DMA queues.

Measured state: 403,058ns = ~31us head (first exp; scalar's DMA ring
must drain its input triggers first — a trigger on a full ring blocks
the queue at transfer-completion pace, so scalar only gets triggers
through xkv-tb1) + ~347us window (ACT 288us busy; ~22us j0 idle is
V-production PE capacity + remainder-DMA pacing, ~12us hp3 is po
overload, ~5us at head-pair boundaries from the normalize/exchange
chain) + ~25us tail-to-drain.

Measured dead ends, do not retry: exp FD=2048 (PSUM bank budget: 4
score banks + 2 PV + 2 projection banks is exactly 8; bf16 matmul
output is TRN3-only), PE warm-up matmuls, ACT-table preload, batched
3D strided input DMAs (device-fatal descriptor fault),
reciprocal_approx_fast off partition 0 (silently wrong),
high_priority offset 96 (knots vector/gpsimd block-end chains, 15us
boundary stalls), ex ring < 8 bufs (-65us!), po stores on gpsimd,
po(3) aog prefetch hoisted to j1 with aog bufs 24 (SBUF + pool-ring
conflicts; regressed to 471us with ex=6).  Run-to-run noise is
+-10-20us from chip power states.
"""

import numpy as np

import concourse.bacc as bacc
import concourse.mybir as mybir
import concourse.tile as tile
from concourse import bass_utils

try:
    import ml_dtypes
    _BF16 = ml_dtypes.bfloat16
except ImportError:  # pragma: no cover
    _BF16 = mybir.dt.np(mybir.dt.bfloat16)

N_CORES = 8
P = 128
TQ = 2048
TKV = 2048
CQ = 1024     # q_tokens channels
CKV = 768     # kv_tokens channels
DQ = 512      # per-core head-group q dims (8 heads x 64)
DO = 512      # per-core output channels (half of 1024)
NJ = 4        # 512-wide tq j-blocks (== projection t-blocks)
NI = TKV // P  # 16 kv chunks
NHP = DQ // P  # 4 head-pairs
CQ_CH = CQ // P   # 8
CKV_CH = CKV // P  # 6
NCC = 2 * NHP     # 8 dc chunks in the gathered attention output

F32 = mybir.dt.float32
BF = mybir.dt.bfloat16
EXP = mybir.ActivationFunctionType.Exp
MUL = mybir.AluOpType.mult

_compiled = None


def _build():
    nc = bacc.Bacc("TRN2", target_bir_lowering=False, debug=False,
                   num_devices=N_CORES)

    xqT = nc.dram_tensor("xqT", [CQ, TQ], BF, kind="ExternalInput")
    xkvT = nc.dram_tensor("xkvT", [CKV, TKV], BF, kind="ExternalInput")
    # wq/wk come hp-major from the host: [128, hp, chunk, 128] flattened,
    # so one contiguous transfer delivers exactly one head-pair's slice.
    wqT = nc.dram_tensor("wqT", [P, NHP * CQ_CH * P], BF,
                         kind="ExternalInput")
    wkT = nc.dram_tensor("wkT", [P, NHP * CKV_CH * P], BF,
                         kind="ExternalInput")
    wvT = nc.dram_tensor("wvT", [P, CKV_CH * DQ], BF, kind="ExternalInput")
    # full-dc Wo slice for this core's output-channel half, dc rows in
    # gathered order (head-group 0 rows then head-group 1 rows)
    woT = nc.dram_tensor("woT", [P, NCC * DO], BF, kind="ExternalInput")
    onesc = nc.dram_tensor("onesc", [P, 8], BF, kind="ExternalInput")
    out_ext = nc.dram_tensor("out", [DO, TQ], BF, kind="ExternalOutput")

    groups = [[2 * b, 2 * b + 1] for b in range(N_CORES // 2)]

    with tile.TileContext(nc) as tc:
        with (
            tc.tile_pool(name="weights", bufs=1) as wpool,
            tc.tile_pool(name="xres", bufs=1) as xpool,
            tc.tile_pool(name="kqv", bufs=1) as kpool,
            tc.tile_pool(name="stage", bufs=1) as stpool,
            tc.tile_pool(name="dram", bufs=1, space="DRAM") as dpool,
            tc.tile_pool(name="psum_s", bufs=2, space="PSUM") as ps_s,
            tc.tile_pool(name="psum_pv", bufs=2, space="PSUM") as ps_pv,
            tc.tile_pool(name="psum_pj", bufs=2, space="PSUM") as ps_pj,
        ):
            # ---- resident weights + token inputs (bf16) ----
            wk_sb = wpool.tile([P, NHP, CKV_CH, P], BF, tag="wk")
            wq_sb = wpool.tile([P, NHP, CQ_CH, P], BF, tag="wq")
            wv_sb = wpool.tile([P, CKV_CH, DQ], BF, tag="wv")
            wo_sb = wpool.tile([P, NCC, DO], BF, tag="wo")
            ones_sb = wpool.tile([P, 8, 1], BF, tag="ones")
            xkv_sb = [xpool.tile([P, TKV], BF, tag="xkv", bufs=CKV_CH,
                                 name=f"xkv{c}") for c in range(CKV_CH)]
            xq_sb = [xpool.tile([P, TQ], BF, tag="xq", bufs=CQ_CH,
                                name=f"xq{c}") for c in range(CQ_CH)]

            # All input transfers issued up front on the three DMA-capable
            # queues (SP/gpsimd/ACT — triggers are ~650ns each and all fit
            # in the scalar queue's pre-first-exp idle), in need-by order;
            # per-queue transfers execute in trigger order, so issue order
            # IS arrival order.  Prefix gating the first exp: wk-hp0,
            # xkv-tb0, wv (feeds prelude V), wq-hp0, xq-tb0 = 2.9MB.  Then
            # xkv-tb1 (K(0,1)+V mid-j0), xkv-tb23, xq-tb1 (Q(0,1) by j1),
            # wk/wq hp1-3, xq-tb23 (Q(0,2)+ by j2), wo (head-pair 3).
            q3 = [nc.sync, nc.gpsimd, nc.scalar]
            xfers = [(wk_sb[:, 0], wkT.ap()[:, 0:CKV_CH * P])]
            for c in range(CKV_CH):
                xfers.append((xkv_sb[c][:, 0:512],
                              xkvT.ap()[c * P:(c + 1) * P, 0:512]))
            for c in range(CKV_CH):
                xfers.append((wv_sb[:, c, :],
                              wvT.ap()[:, c * DQ:(c + 1) * DQ]))
            xfers.append((wq_sb[:, 0], wqT.ap()[:, 0:CQ_CH * P]))
            for c in range(CQ_CH):
                xfers.append((xq_sb[c][:, 0:512],
                              xqT.ap()[c * P:(c + 1) * P, 0:512]))
            for c in range(CKV_CH):
                xfers.append((xkv_sb[c][:, 512:1024],
                              xkvT.ap()[c * P:(c + 1) * P, 512:1024]))
            for c in range(CKV_CH):
                xfers.append((xkv_sb[c][:, 1024:TKV],
                              xkvT.ap()[c * P:(c + 1) * P, 1024:TKV]))
            for c in range(CQ_CH):
                xfers.append((xq_sb[c][:, 512:1024],
                              xqT.ap()[c * P:(c + 1) * P, 512:1024]))
            for hp in range(1, NHP):
                xfers.append((wk_sb[:, hp],
                              wkT.ap()[:, hp * CKV_CH * P:(hp + 1) * CKV_CH * P]))
                xfers.append((wq_sb[:, hp],
                              wqT.ap()[:, hp * CQ_CH * P:(hp + 1) * CQ_CH * P]))
            for c in range(CQ_CH):
                xfers.append((xq_sb[c][:, 1024:TQ],
                              xqT.ap()[c * P:(c + 1) * P, 1024:TQ]))
            for cc in range(NCC):
                xfers.append((wo_sb[:, cc, :],
                              woT.ap()[:, cc * DO:(cc + 1) * DO]))
            # ones first (2KB — gates the vt ones-columns for the first PV)
            nc.gpsimd.dma_start(ones_sb[:],
                                onesc.ap().rearrange("p (n o) -> p n o", o=1))
            # A DMA trigger BLOCKS its engine queue while the hardware ring
            # is full, pacing at transfer-completion rate — so the scalar
            # queue only gets triggers that drain before the first exp
            # (~5 x 128-192KB); everything later goes to sync/gpsimd.
            # (three queues through xkv-tb1: scalar's ring drains those
            # triggers before the first exp, and j0's inline K/V work
            # otherwise starves on the 2-queue remainder stream)
            for n, (dst, src) in enumerate(xfers):
                if n < 28:
                    q3[n % 3].dma_start(dst, src)
                else:
                    q3[n % 2].dma_start(dst, src)

            # ---- SBUF-resident K/Q/V (written by projection evictions) ----
            kb = [[kpool.tile([P, 512], BF, tag="kb", bufs=NHP * NJ,
                              name=f"kb{hp}_{tb}") for tb in range(NJ)]
                  for hp in range(NHP)]
            qs = [[kpool.tile([P, 512], BF, tag="qs", bufs=NHP * NJ,
                              name=f"qs{hp}_{tb}") for tb in range(NJ)]
                  for hp in range(NHP)]
            # vt[tc]: [128 tkv-chunk, 8 heads, 64+1] (ones col -> denominator)
            vt = [kpool.tile([P, 8, 65], BF, tag="vt", bufs=NI,
                             name=f"vt{tc}") for tc in range(NI)]
            # normalized attention output per head-pair (exchanged via CC)
            ao = [kpool.tile([P, TQ], BF, tag="ao", bufs=NHP,
                             name=f"ao{hp}") for hp in range(NHP)]

            # ---- internal DRAM for collectives ----
            agi = [dpool.tile([P, TQ], BF, tag=f"agi{h}", name=f"agi{h}")
                   for h in range(NHP - 1)]
            ago = [dpool.tile([2, P, TQ], BF, tag=f"ago{h}", name=f"ago{h}")
                   for h in range(NHP - 1)]
            agi3 = [dpool.tile([P, 512], BF, tag=f"agi3_{j}", name=f"agi3_{j}")
                    for j in range(NJ)]
            ago3 = [dpool.tile([2, P, 512], BF, tag=f"ago3_{j}",
                               name=f"ago3_{j}") for j in range(NJ)]

            # ============ projection work units (one yield per MM) ========
            def k_gen(hp, tb):
                pk = ps_pj.tile([P, 512], F32, tag="pj", name=f"pk_{hp}_{tb}")
                for c in range(CKV_CH):
                    with tc.high_priority(offset=-64):
                        nc.tensor.matmul(
                            pk[:], wk_sb[:, hp, c, :],
                            xkv_sb[c][:, tb * 512:(tb + 1) * 512],
                            start=(c == 0), stop=(c == CKV_CH - 1))
                    if c == CKV_CH - 1:
                        nc.vector.tensor_copy(kb[hp][tb][:], pk[:])
                    yield

            def q_gen(hp, tb):
                pq = ps_pj.tile([P, 512], F32, tag="pj", name=f"pq_{hp}_{tb}")
                for c in range(CQ_CH):
                    with tc.high_priority(offset=-64):
                        nc.tensor.matmul(
                            pq[:], wq_sb[:, hp, c, :],
                            xq_sb[c][:, tb * 512:(tb + 1) * 512],
                            start=(c == 0), stop=(c == CQ_CH - 1))
                    if c == CQ_CH - 1:
                        nc.vector.tensor_copy(qs[hp][tb][:], pq[:])
                    yield

            def v_gen(tc_i):
                pv = ps_pj.tile([P, 512], F32, tag="pj", name=f"pv_{tc_i}")
                for c in range(CKV_CH):
                    with tc.high_priority(offset=-64):
                        # commit-late like the po matmuls: these wait on
                        # streaming xkv/weight arrivals, and committed early
                        # they stall QK pairs behind them in the PE FIFO
                        nc.tensor.matmul(
                            pv[:],
                            xkv_sb[c][:, tc_i * P:(tc_i + 1) * P],
                            wv_sb[:, c, :],
                            start=(c == 0), stop=(c == CKV_CH - 1))
                    if c == CKV_CH - 1:
                        nc.vector.tensor_copy(
                            vt[tc_i][:, :, 0:64],
                            pv[:].rearrange("p (h d) -> p h d", d=64))
                        nc.vector.tensor_copy(vt[tc_i][:, :, 64:65],
                                              ones_sb[:])
                    yield

            def run_all(gen):
                for _ in gen:
                    pass

            # deferred projection work, drip-fed into the attention loop.
            # Order respects need-by times: Q(hp,tb) before block (hp,tb)
            # starts, K(hp) fully before head-pair hp starts.
            deferred = [
                q_gen(0, 2),
                k_gen(1, 0), k_gen(1, 1),
                q_gen(0, 3),
                k_gen(1, 2), k_gen(1, 3),
                q_gen(1, 0), q_gen(1, 1),
                k_gen(2, 0), k_gen(2, 1), k_gen(2, 2), k_gen(2, 3),
                q_gen(1, 2), q_gen(1, 3),
                q_gen(2, 0),
                k_gen(3, 0), k_gen(3, 1),
                q_gen(2, 1), q_gen(2, 2),
                k_gen(3, 2), k_gen(3, 3),
                q_gen(2, 3),
                q_gen(3, 0), q_gen(3, 1), q_gen(3, 2), q_gen(3, 3),
            ]
            deferred.reverse()

            def pop_work(queue, n):
                while n > 0 and queue:
                    gen = queue[-1]
                    try:
                        next(gen)
                        n -= 1
                    except StopIteration:
                        queue.pop()

            # ================= prelude =================
            # K(0,0) + V(0..3) + Q(0,0).  V production fills the PE-idle
            # DMA-wait window (wv+xkv-tb0 arrive well before wq/xq-tb0);
            # the scheduler lets Q00 overtake any V matmul still waiting
            # on data.  K(0,1..3) and V(4..15) are produced inside j0's
            # iterations as their chunks land; Q(0,1) late in j0.
            run_all(k_gen(0, 0))
            for t in range(4):
                run_all(v_gen(t))
            run_all(q_gen(0, 0))
            k0 = {2: [k_gen(0, 1)], 6: [k_gen(0, 2)], 10: [k_gen(0, 3)]}
            q01 = [q_gen(0, 1)]

            # ============== out-projection work units ==============
            # Chunk order per chain: head-pairs 0-2 first (gathered long
            # ago, DMA'd with no wait), head-pair 3's pair last so only 2
            # of 8 matmuls gate on the final per-j exchange.
            po_order = [(g, hpx) for hpx in range(NHP) for g in range(2)]
            # scalar stays exp-only until the tail, and gpsimd carries the
            # collectives + partition broadcasts — po traffic goes on sync
            # (j=3's stores can use scalar once the exp stream has drained)
            ld_rot = [nc.sync, nc.sync]

            def po_loads(j, pre):
                js = slice(j * 512, (j + 1) * 512)
                for n, (g, hpx) in enumerate(po_order):
                    if hpx == NHP - 1:
                        continue
                    aog = stpool.tile([P, 512], BF, tag="aog", bufs=16,
                                      name=f"aog_{j}_{g}_{hpx}")
                    ld_rot[n % 2].dma_start(aog[:], ago[hpx][g, :, js])
                    pre[n] = aog

            def po_gen(j, pre):
                tail = j == NJ - 1
                for n, (g, hpx) in enumerate(po_order):
                    if hpx == NHP - 1:
                        aog = stpool.tile([P, 512], BF, tag="aog", bufs=16,
                                          name=f"aog_{j}_{g}_{hpx}")
                        eng = nc.scalar if tail else ld_rot[n % 2]
                        eng.dma_start(aog[:], ago3[j][g, :, :])
                        pre[n] = aog
                js = slice(j * 512, (j + 1) * 512)
                for do in range(DO // P):
                    po = ps_pj.tile([P, 512], F32, tag="pj",
                                    name=f"po_{j}_{do}")
                    for n, (g, hpx) in enumerate(po_order):
                        cc = g * NHP + hpx
                        # commit late in the tensor FIFO (negative boost):
                        # these wait on ring-paced aog loads, and committed
                        # early they stall QK pairs behind them (measured
                        # 6-8us exp gaps in the j3 drip region)
                        with tc.high_priority(offset=-64):
                            nc.tensor.matmul(
                                po[:], wo_sb[:, cc, do * P:(do + 1) * P],
                                pre[n][:],
                                start=(n == 0), stop=(n == NCC - 1))
                        yield
                    ost = stpool.tile([P, 512], BF, tag="ost", bufs=3,
                                      name=f"ost_{j}_{do}")
                    nc.vector.tensor_copy(ost[:], po[:])
                    oeng = (nc.scalar if tail and do % 3 == 2
                            else ld_rot[do % 2])
                    oeng.dma_start(out_ext[do * P:(do + 1) * P, js],
                                   ost[:])

            po_pre = {j: [None] * NCC for j in range(NJ)}
            po_work = []

            # ================= attention =================
            for hp in range(NHP):
                for j in range(NJ):
                    js = slice(j * 512, (j + 1) * 512)
                    acc_a = ps_pv.tile([P, 512], F32, tag="pv")
                    acc_b = ps_pv.tile([P, 512], F32, tag="pv")
                    if hp == NHP - 1 and j == NJ - 1:
                        # prefetch j=3's six early out-projection inputs
                        po_loads(NJ - 1, po_pre[NJ - 1])
                    for i in range(NI):
                        tbk, ik = i // 4, i % 4
                        ks = slice(ik * 128, (ik + 1) * 128)
                        sc = ps_s.tile([P, 1024], F32, tag="sc")
                        # the exp stream lives or dies by these two being
                        # scheduled the moment their PSUM buffer frees; the
                        # boost must stay local (~1 iteration) — offset 96
                        # let attention race ahead of the block-end
                        # normalize/exchange chains and knotted the vector
                        # and gpsimd queues at head-pair boundaries
                        with tc.high_priority(offset=16):
                            nc.tensor.matmul(sc[:, 0:512],
                                             kb[hp][tbk][0:64, ks],
                                             qs[hp][j][0:64, :],
                                             start=True, stop=True)
                            nc.tensor.matmul(sc[:, 512:1024],
                                             kb[hp][tbk][64:128, ks],
                                             qs[hp][j][64:128, :],
                                             start=True, stop=True)
                        ex = stpool.tile([P, 1024], BF, tag="ex", bufs=8)
                        nc.scalar.activation(ex[:], sc[:], EXP, scale=0.125)
                        # first block: K(0,tb) and V chunk i are produced
                        # inline as their tb-blocks land from DRAM; Q(0,1)
                        # lands in the later iterations, in time for j1.
                        if hp == 0 and j == 0:
                            if i in k0:
                                run_all(k0[i][0])
                            if i >= 4:
                                run_all(v_gen(i))
                            if i >= 10:
                                pop_work(q01, 2)
                        nc.tensor.matmul(acc_a[0:65, :],
                                         vt[i][:, 2 * hp, :],
                                         ex[:, 0:512],
                                         start=(i == 0), stop=(i == NI - 1))
                        nc.tensor.matmul(acc_b[0:65, :],
                                         vt[i][:, 2 * hp + 1, :],
                                         ex[:, 512:1024],
                                         start=(i == 0), stop=(i == NI - 1))
                        if not (hp == 0 and j == 0):
                            pop_work(deferred, 2 if j == 3 else 1)
                        # drip the out projection into head-pair 3
                        if hp == NHP - 1 and j >= 1:
                            pop_work(po_work, 2)
                    # evict BOTH accumulators first (frees the PSUM ring for
                    # the next j-block before the slow reciprocals run),
                    # then normalize: ao[:, js] = acc[0:64] / acc[64]
                    pvsts, recs, bcs = [], [], []
                    for half, acc in ((0, acc_a), (1, acc_b)):
                        pvst = stpool.tile([P, 512], F32, tag="pvst", bufs=4,
                                           name=f"pvst_{hp}_{j}_{half}")
                        nc.vector.tensor_copy(pvst[0:65, :], acc[0:65, :])
                        pvsts.append(pvst)
                    for half in (0, 1):
                        # reciprocal_approx_fast is only correct with
                        # partition-0 operands (verified on hw), so hop the
                        # denominator row down first
                        den = stpool.tile([P, 512], F32, tag="den", bufs=2)
                        nc.vector.tensor_copy(den[0:1, :],
                                              pvsts[half][64:65, :])
                        rec = stpool.tile([P, 512], F32, tag="rec", bufs=2)
                        nc.vector.reciprocal_approx_fast(rec[0:1, :],
                                                         den[0:1, :])
                        recs.append(rec)
                        bc = stpool.tile([P, 512], F32, tag="bc", bufs=2)
                        nc.gpsimd.partition_broadcast(bc[0:64, :],
                                                      rec[0:1, :],
                                                      channels=64)
                        bcs.append(bc)
                    for half in (0, 1):
                        nc.vector.tensor_tensor(
                            ao[hp][half * 64:(half + 1) * 64, js],
                            pvsts[half][0:64, :], bcs[half][0:64, :], op=MUL)
                    # exchanges: head-pairs 0-2 once per hp (overlapped with
                    # the next head-pair); head-pair 3 per j-block so the
                    # output projection can start before attention ends.
                    if hp == NHP - 1:
                        # final block's exchange input skips sync's ring
                        # backlog; scalar is idle after the last exp issues
                        aeng = nc.scalar if j == NJ - 1 else nc.sync
                        aeng.dma_start(agi3[j][:], ao[hp][:, js])
                        nc.gpsimd.collective_compute(
                            "AllGather", mybir.AluOpType.bypass,
                            replica_groups=groups,
                            ins=[agi3[j].opt()], outs=[ago3[j].opt()])
                        if j < NJ - 1:
                            po_loads(j, po_pre[j])
                            po_work.insert(0, po_gen(j, po_pre[j]))
                if hp < NHP - 1:
                    nc.sync.dma_start(agi[hp][:], ao[hp][:])
                    nc.gpsimd.collective_compute(
                        "AllGather", mybir.AluOpType.bypass,
                        replica_groups=groups,
                        ins=[agi[hp].opt()], outs=[ago[hp].opt()])

            # ===== output projection tail =====
            for gen in reversed(po_work):
                run_all(gen)
            run_all(po_gen(NJ - 1, po_pre[NJ - 1]))

    nc.compile()
    return nc


def make_in_maps(q_tokens, kv_tokens, Wq, Wk, Wv, Wo):
    q_tokens = np.asarray(q_tokens, np.float32)
    kv_tokens = np.asarray(kv_tokens, np.float32)
    Wq = np.asarray(Wq, np.float32)
    Wk = np.asarray(Wk, np.float32)
    Wv = np.asarray(Wv, np.float32)
    Wo = np.asarray(Wo, np.float32)

    def chunked(w):
        # [in, out] -> [128, n_chunks*out]: contiguous per-partition image
        # of the SBUF-resident [P, n, out] weight tiles
        n = w.shape[0] // P
        return np.ascontiguousarray(
            w.reshape(n, P, w.shape[1]).transpose(1, 0, 2).reshape(P, -1)
        ).astype(_BF16)

    def chunked_hp(w):
        # [in, 512] -> [128, hp, chunk, 128] image so one contiguous
        # transfer delivers one head-pair's slice of every input chunk
        n = w.shape[0] // P
        t = w.reshape(n, P, NHP, P)           # [chunk, p, hp, d]
        return np.ascontiguousarray(
            t.transpose(1, 2, 0, 3).reshape(P, -1)
        ).astype(_BF16)

    in_maps = []
    for c in range(N_CORES):
        b, hg = c // 2, c % 2
        sl = slice(hg * DQ, (hg + 1) * DQ)
        osl = slice(hg * DO, (hg + 1) * DO)
        in_maps.append({
            "xqT": np.ascontiguousarray(q_tokens[b].T).astype(_BF16),
            "xkvT": np.ascontiguousarray(kv_tokens[b].T).astype(_BF16),
            "wqT": chunked_hp(Wq[sl, :].T),
            "wkT": chunked_hp(Wk[sl, :].T),
            "wvT": chunked(Wv[sl, :].T),
            # [dc, do-half] with dc rows in gathered (global head) order
            "woT": chunked(Wo[osl, :].T),
            "onesc": np.ones((P, 8), _BF16),
        })
    return in_maps


def kernel(q_tokens, kv_tokens, Wq, Wk, Wv, Wo):
    global _compiled
    if _compiled is None:
        _compiled = _build()
    nc = _compiled

    in_maps = make_in_maps(q_tokens, kv_tokens, Wq, Wk, Wv, Wo)
    res = bass_utils.run_bass_kernel_spmd(nc, in_maps,
                                          core_ids=list(range(N_CORES)))
    B = 4
    out = np.empty((B, TQ, 2 * DO), np.float32)
    for c in range(N_CORES):
        b, hg = c // 2, c % 2
        out[b, :, hg * DO:(hg + 1) * DO] = \
            np.asarray(res.results[c]["out"], np.float32).T
    return out
